# revision 5
# baseline (speedup 1.0000x reference)
"""Trainium2 Bass kernel for nn_PoissonNLLLoss (B=16, H=1024, W=2048, MAX_ID=356).

Computes  LOSS_WEIGHT * (mean(exp(logits)) - inst)  where inst is the mean over
images of the sum of logits gathered at per-segment centroids (segments are
label ids > 100), exactly matching the jax reference semantics.

Sharding: data-parallel over the batch — 2 images per NeuronCore across 8
cores (SPMD, identical program). Host combines the per-core partial scalars
(exp-sum and per-image instance sums), the only cross-core communication.

Per-core algorithm (all segment statistics are exact f32 integer arithmetic):
  id = 32*hi + lo. The image is processed in row-bands of 128 rows; within a
  band each column c is one "chunk" of 128 pixels (partition p = row in band).
  For each chunk one bf16 matmul accumulates into PSUM:
      psum[(s,j), i] += sum_p stat[p,(s,j)] * onehot_lo[p,i]
  with stationary stat = onehot_hi (x) {1, p, c mod 256} — all bf16-exact
  values — grouped per (band, 256-column octant). PSUM evacuations apply the
  exact f32 corrections  sy += 128*band*cnt_g  and  sx += 256*oct*cnt_g.
  One-hots are built on DVE/GPSIMD/ACT in transposed step-1 bf16 layouts
  (DVE 2x packed mode); exp+row-sum rides on ACT via accum_out.
  Finalize on device: centroids via exact floor division (reciprocal +/-1
  correction), indirect-DMA gather of logits at centroid offsets, validity
  masking, and partition reduction via a ones-matmul.
"""

import numpy as np

P = 128
NLO = 32
NHI = 12
NST = 3          # stationary stats {1, p, c mod 256}
MAX_ID = 356
NID = NLO * NHI  # 384 (ids >= 356 never occur -> cnt 0, masked)
OCT = 256        # column span of one PSUM accumulation group
NBLK = 5         # bounce blocks per image: cnt, Sp, Sc, corr_y, corr_x

B, H, W = 16, 1024, 2048
N_CORES = 8
NIMG = B // N_CORES


def _build_nc(n_img, H, W, G=128, trunc_cast=False):
    # trunc_cast: CoreSim truncates on f32->i32 copy; TRN2 HW rounds to
    # nearest. The hi-digit extraction bias must match the cast mode.
    cast_bias = 0.5 if trunc_cast else -15.5
    import concourse.bass as bass
    import concourse.bacc as bacc
    import concourse.tile as tile
    from concourse import mybir

    f32 = mybir.dt.float32
    i32 = mybir.dt.int32
    bf16 = mybir.dt.bfloat16
    Alu = mybir.AluOpType
    Act = mybir.ActivationFunctionType

    NB = H // P
    NOCT = max(W // OCT, 1)
    G = min(G, W)
    NBATCH = W // G
    BPO = max(NBATCH // NOCT, 1)
    M = NST * NHI
    n_btiles = n_img * NB

    nc = bacc.Bacc('TRN2', target_bir_lowering=False, debug=False)
    logits_h = nc.declare_dram_parameter("logits", [n_img, H, W], f32, isOutput=False)
    label_h = nc.declare_dram_parameter("label", [n_img, H, W], i32, isOutput=False)
    out_h = nc.declare_dram_parameter("out", [1, 4], f32, isOutput=True)
    bounce_h = nc.dram_tensor("bounce", [n_img * NBLK * NID], f32)

    with tile.TileContext(nc) as tc:
        import contextlib
        ctx = contextlib.ExitStack()
        with ctx:
            cpool = ctx.enter_context(tc.tile_pool(name="consts", bufs=1))
            bandA = ctx.enter_context(tc.tile_pool(name="bandA", bufs=3))
            bandB = ctx.enter_context(tc.tile_pool(name="bandB", bufs=3))
            batchp = ctx.enter_context(tc.tile_pool(name="batchp", bufs=4))
            accp = ctx.enter_context(tc.tile_pool(name="acc", bufs=1))
            psum = ctx.enter_context(tc.tile_pool(name="psum", bufs=4, space="PSUM"))
            fin = ctx.enter_context(tc.tile_pool(name="fin", bufs=1))

            # ---- constants (transposed step-1 bf16 layouts; values bf16-exact)
            iota32_t = cpool.tile([P, NLO * G], bf16)
            nc.gpsimd.iota(iota32_t[:].rearrange("p (i c) -> p i c", i=NLO),
                           pattern=[[1, NLO], [0, G]], base=0, channel_multiplier=0,
                           allow_small_or_imprecise_dtypes=True)
            iota12_t = cpool.tile([P, NHI * G], bf16)
            nc.gpsimd.iota(iota12_t[:].rearrange("p (j c) -> p j c", j=NHI),
                           pattern=[[1, NHI], [0, G]], base=0, channel_multiplier=0,
                           allow_small_or_imprecise_dtypes=True)
            OCTW = min(OCT, W)
            xr_t = cpool.tile([P, NHI * OCTW], bf16)
            nc.gpsimd.iota(xr_t[:].rearrange("p (j c) -> p j c", j=NHI),
                           pattern=[[0, NHI], [1, OCTW]], base=0, channel_multiplier=0,
                           allow_small_or_imprecise_dtypes=True)
            p_col = cpool.tile([P, 1], f32)
            nc.gpsimd.iota(p_col[:], pattern=[[0, 1]], base=0, channel_multiplier=1,
                           allow_small_or_imprecise_dtypes=True)
            ones_col = cpool.tile([P, 1], f32)
            nc.vector.memset(ones_col[:], 1.0)
            # id layout after bounce reload: id = 3*p + j at [p, img*3 + j]
            idf = cpool.tile([P, n_img * 3], f32)
            nc.gpsimd.iota(idf[:].rearrange("p (g i) -> p g i", g=n_img),
                           pattern=[[0, n_img], [1, 3]], base=0,
                           channel_multiplier=3,
                           allow_small_or_imprecise_dtypes=True)

            exp_accs = accp.tile([P, n_btiles], f32)
            octw = cpool.tile([P, NOCT], f32)
            nc.gpsimd.iota(octw[:], pattern=[[1, NOCT]], base=0,
                           channel_multiplier=0,
                           allow_small_or_imprecise_dtypes=True)
            nc.vector.tensor_scalar(out=octw[:], in0=octw[:], scalar1=float(OCT),
                                    scalar2=None, op0=Alu.mult)
            xw_scr = accp.tile([NHI, NOCT * NLO], f32)
            xw_dummy = accp.tile([NHI, 1], f32)

            accs = []
            for img in range(n_img):
                a = accp.tile([M, NLO], f32, tag=f"acc{img}")
                cy = accp.tile([NHI, NLO], f32, tag=f"accy{img}")
                cx = accp.tile([NHI, NLO], f32, tag=f"accx{img}")
                nc.vector.memset(a[:], 0.0)
                nc.vector.memset(cy[:], 0.0)
                nc.vector.memset(cx[:], 0.0)
                accs.append((a, cy, cx))

            for img in range(n_img):
                acc, acc2y, acc2x = accs[img]
                for band in range(NB):
                    r0 = band * P
                    label_band = bandA.tile([P, W], i32, tag="label_band")
                    nc.gpsimd.dma_start(out=label_band[:], in_=label_h[img, r0:r0 + P, :])
                    logits_band = bandA.tile([P, W], f32, tag="logits_band")
                    nc.gpsimd.dma_start(out=logits_band[:], in_=logits_h[img, r0:r0 + P, :])

                    # exp + per-partition row-sum fused on ACT
                    exp_scr = bandB.tile([P, W], bf16, tag="exp_scr")
                    nc.scalar.activation(
                        out=exp_scr[:], in_=logits_band[:], func=Act.Exp,
                        accum_out=exp_accs[:, img * NB + band: img * NB + band + 1])

                    # hi = int_cast((label + cast_bias)/32); lo = label - 32*hi
                    hi_i = bandB.tile([P, W], i32, tag="hi_i")
                    nc.vector.tensor_scalar(out=hi_i[:], in0=label_band[:],
                                            scalar1=cast_bias, scalar2=1.0 / NLO,
                                            op0=Alu.add, op1=Alu.mult)
                    hi_bf = bandB.tile([P, W], bf16, tag="hi_bf")
                    nc.scalar.activation(out=hi_bf[:], in_=hi_i[:], func=Act.Copy)
                    lo_bf = bandB.tile([P, W], bf16, tag="lo_bf")
                    nc.vector.scalar_tensor_tensor(out=lo_bf[:], in0=hi_bf[:],
                                                   scalar=-float(NLO), in1=label_band[:],
                                                   op0=Alu.mult, op1=Alu.add)

                    ps = psum.tile([M, NOCT * NLO], f32, tag="psband")
                    for oct_i in range(NOCT):
                        for bj in range(BPO):
                            bi = oct_i * BPO + bj
                            c0 = bi * G
                            # transposed layouts: innermost dim = chunk (step 1)
                            alo = batchp.tile([P, NLO * G], bf16, tag="alo")
                            alo_v = alo[:].rearrange("p (i c) -> p i c", i=NLO)
                            lo_b = lo_bf[:, c0:c0 + G].unsqueeze(1).to_broadcast([P, NLO, G])
                            nc.vector.tensor_tensor(
                                out=alo_v, in0=lo_b,
                                in1=iota32_t[:].rearrange("p (i c) -> p i c", i=NLO),
                                op=Alu.is_equal)

                            stat = batchp.tile([P, M * G], bf16, tag="stat")
                            stat_v = stat[:].rearrange("p (s j c) -> p s j c", s=NST, j=NHI)
                            hi_b = hi_bf[:, c0:c0 + G].unsqueeze(1).to_broadcast([P, NHI, G])
                            nc.vector.tensor_tensor(
                                out=stat_v[:, 0, :, :], in0=hi_b,
                                in1=iota12_t[:].rearrange("p (j c) -> p j c", j=NHI),
                                op=Alu.is_equal)
                            # stat * p on ACT (per-partition scale)
                            nc.scalar.activation(out=stat_v[:, 1, :, :],
                                                 in_=stat_v[:, 0, :, :],
                                                 func=Act.Copy, scale=p_col[:, 0:1])
                            # stat * (c mod 256) on GPSIMD
                            xr_sl = xr_t[:].rearrange("p (j c) -> p j c", j=NHI)[
                                :, :, bj * G:(bj + 1) * G]
                            nc.gpsimd.tensor_tensor(out=stat_v[:, 2, :, :],
                                                    in0=stat_v[:, 0, :, :],
                                                    in1=xr_sl, op=Alu.mult)

                            for g in range(G):
                                nc.tensor.matmul(
                                    out=ps[:, oct_i * NLO:(oct_i + 1) * NLO],
                                    lhsT=stat_v[:, :, :, g],
                                    rhs=alo_v[:, :, g],
                                    start=(bj == 0 and g == 0),
                                    stop=(bj == BPO - 1 and g == G - 1),
                                )

                    # evacuate band: band-total = sum over octants (exact f32);
                    # corr_y += 128*band*cnt_band; corr_x += 256*sum_oct oct*cnt_oct
                    ps_v = ps[:].rearrange("m (o i) -> m i o", o=NOCT)
                    bsum = bandB.tile([M, NLO], f32, tag="bsum")
                    nc.vector.tensor_reduce(out=bsum[:], in_=ps_v,
                                            axis=mybir.AxisListType.X, op=Alu.add)
                    nc.vector.tensor_tensor(out=acc[:], in0=acc[:], in1=bsum[:],
                                            op=Alu.add)
                    if band:
                        nc.vector.scalar_tensor_tensor(
                            out=acc2y[:], in0=bsum[0:NHI, :], scalar=float(P * band),
                            in1=acc2y[:], op0=Alu.mult, op1=Alu.add)
                    xw = bandB.tile([NHI, NLO], f32, tag="xw")
                    nc.vector.tensor_tensor(
                        out=xw_scr[:].rearrange("m (o i) -> m i o", o=NOCT),
                        in0=ps_v[0:NHI], in1=octw[0:NHI].unsqueeze(1).to_broadcast(
                            [NHI, NLO, NOCT]),
                        op=Alu.mult)
                    nc.vector.tensor_reduce(
                        out=xw[:], in_=xw_scr[:].rearrange("m (o i) -> m i o", o=NOCT),
                        axis=mybir.AxisListType.X, op=Alu.add)
                    nc.vector.tensor_tensor(out=acc2x[:], in0=acc2x[:], in1=xw[:],
                                            op=Alu.add)

            # ---- finalize ----
            for img in range(n_img):
                acc, acc2y, acc2x = accs[img]
                base = img * NBLK * NID
                nc.gpsimd.dma_start(
                    out=bounce_h[base:base + 3 * NID].rearrange("(p c) -> p c", p=M),
                    in_=acc[:])
                nc.gpsimd.dma_start(
                    out=bounce_h[base + 3 * NID:base + 4 * NID]
                    .rearrange("(p c) -> p c", p=NHI), in_=acc2y[:])
                nc.gpsimd.dma_start(
                    out=bounce_h[base + 4 * NID:base + 5 * NID]
                    .rearrange("(p c) -> p c", p=NHI), in_=acc2x[:])

            def reload(s):
                t = fin.tile([P, n_img * 3], f32, tag=f"re{s}")
                src = bounce_h[:].rearrange("(i s p j) -> p i s j", i=n_img, s=NBLK, p=P)
                nc.gpsimd.dma_start(out=t[:].rearrange("p (i j) -> p i j", i=n_img),
                                    in_=src[:, :, s, :])
                return t

            cnt = reload(0)
            sy = reload(1)
            sx = reload(2)
            cry = reload(3)
            crx = reload(4)
            Alu_ = Alu
            nc.vector.tensor_tensor(out=sy[:], in0=sy[:], in1=cry[:], op=Alu_.add)
            nc.vector.tensor_tensor(out=sx[:], in0=sx[:], in1=crx[:], op=Alu_.add)

            denom = fin.tile([P, n_img * 3], f32, tag="denom")
            nc.vector.tensor_scalar(out=denom[:], in0=cnt[:], scalar1=1.0, scalar2=None,
                                    op0=Alu.max)
            rcp = fin.tile([P, n_img * 3], f32, tag="rcp")
            nc.vector.reciprocal(rcp[:], denom[:])

            def floordiv(s_t, nm):
                # exact floor(s/denom): approximate quotient then +/-1 fix,
                # insensitive to the f32->i32 cast rounding mode
                qf = fin.tile([P, n_img * 3], f32, tag=f"qf{nm}")
                nc.vector.tensor_tensor(out=qf[:], in0=s_t[:], in1=rcp[:], op=Alu.mult)
                qi = fin.tile([P, n_img * 3], i32, tag=f"qi{nm}")
                nc.vector.tensor_copy(qi[:], qf[:])
                q = fin.tile([P, n_img * 3], f32, tag=f"q{nm}")
                nc.vector.tensor_copy(q[:], qi[:])
                r = fin.tile([P, n_img * 3], f32, tag=f"r{nm}")
                nc.vector.tensor_tensor(out=r[:], in0=q[:], in1=denom[:], op=Alu.mult)
                nc.vector.tensor_tensor(out=r[:], in0=s_t[:], in1=r[:], op=Alu.subtract)
                corr = fin.tile([P, n_img * 3], f32, tag=f"corr{nm}")
                nc.vector.tensor_tensor(out=corr[:], in0=r[:], in1=denom[:], op=Alu.is_ge)
                nc.vector.tensor_tensor(out=q[:], in0=q[:], in1=corr[:], op=Alu.add)
                nc.vector.tensor_scalar(out=corr[:], in0=r[:], scalar1=0.0, scalar2=None,
                                        op0=Alu.is_lt)
                nc.vector.tensor_tensor(out=q[:], in0=q[:], in1=corr[:], op=Alu.subtract)
                return q

            qy = floordiv(sy, "y")
            qx = floordiv(sx, "x")

            offs_f = fin.tile([P, n_img * 3], f32, tag="offs_f")
            nc.vector.scalar_tensor_tensor(out=offs_f[:], in0=qy[:], scalar=float(W),
                                           in1=qx[:], op0=Alu.mult, op1=Alu.add)
            mask = fin.tile([P, n_img * 3], f32, tag="mask")
            nc.vector.tensor_scalar(out=mask[:], in0=idf[:], scalar1=100.0, scalar2=None,
                                    op0=Alu.is_gt)
            m2 = fin.tile([P, n_img * 3], f32, tag="m2")
            nc.vector.tensor_scalar(out=m2[:], in0=cnt[:], scalar1=0.0, scalar2=None,
                                    op0=Alu.is_gt)
            nc.vector.tensor_tensor(out=mask[:], in0=mask[:], in1=m2[:], op=Alu.mult)
            nc.vector.tensor_tensor(out=offs_f[:], in0=offs_f[:], in1=mask[:], op=Alu.mult)
            offs_i = fin.tile([P, n_img * 3], i32, tag="offs_i")
            nc.vector.tensor_copy(offs_i[:], offs_f[:])

            # gather logits at centroids (one offset per partition per DMA)
            gath = fin.tile([P, n_img * 3], f32, tag="gath")
            for img in range(n_img):
                for j in range(3):
                    col = img * 3 + j
                    nc.gpsimd.indirect_dma_start(
                        out=gath[:, col:col + 1],
                        out_offset=None,
                        in_=logits_h[:].rearrange("i h w -> (i h w)").unsqueeze(1),
                        in_offset=bass.IndirectOffsetOnAxis(
                            ap=offs_i[:, col:col + 1], axis=0),
                        element_offset=img * H * W,
                    )

            nc.vector.tensor_tensor(out=gath[:], in0=gath[:], in1=mask[:], op=Alu.mult)

            red = fin.tile([P, n_img + 1], f32, tag="red")
            for img in range(n_img):
                nc.vector.tensor_reduce(out=red[:, img:img + 1],
                                        in_=gath[:, img * 3:(img + 1) * 3],
                                        axis=mybir.AxisListType.X, op=Alu.add)
            nc.vector.tensor_reduce(out=red[:, n_img:n_img + 1], in_=exp_accs[:],
                                    axis=mybir.AxisListType.X, op=Alu.add)

            ps_fin = psum.tile([1, n_img + 1], f32, tag="ps_fin")
            nc.tensor.matmul(out=ps_fin[:], lhsT=ones_col[:], rhs=red[:],
                             start=True, stop=True)

            out_sb = fin.tile([1, 4], f32, tag="out_sb")
            nc.vector.memset(out_sb[:], 0.0)
            nc.vector.tensor_copy(out_sb[:, 0:1], ps_fin[:, n_img:n_img + 1])
            for img in range(n_img):
                nc.vector.tensor_copy(out_sb[:, 1 + img:2 + img], ps_fin[:, img:img + 1])
            nc.gpsimd.dma_start(out=out_h[:], in_=out_sb[:])

    nc.compile()
    return nc


_NC_CACHE = {}


def kernel(logits, label):
    logits = np.ascontiguousarray(np.asarray(logits, dtype=np.float32))
    label = np.ascontiguousarray(np.asarray(label, dtype=np.int32))
    assert logits.shape == (B, H, W), logits.shape
    assert label.shape == (B, H, W), label.shape

    from concourse.bass_utils import run_bass_kernel_spmd

    key = (NIMG, H, W)
    if key not in _NC_CACHE:
        _NC_CACHE[key] = _build_nc(NIMG, H, W, G=128)
    nc = _NC_CACHE[key]

    in_maps = [
        {"logits": logits[c * NIMG:(c + 1) * NIMG],
         "label": label[c * NIMG:(c + 1) * NIMG]}
        for c in range(N_CORES)
    ]
    # the axon-proxied device occasionally reports a transient
    # NRT_EXEC_UNIT_UNRECOVERABLE; retry a few times before giving up
    import time as _time
    last_exc = None
    for attempt in range(4):
        try:
            res = run_bass_kernel_spmd(nc, in_maps, list(range(N_CORES)))
            break
        except Exception as e:  # jax.errors.JaxRuntimeError and friends
            last_exc = e
            _time.sleep(2.0 * (attempt + 1))
    else:
        raise last_exc

    # host-side combine of the per-core partial scalars (the two "all-reduces")
    exp_total = 0.0
    inst_total = 0.0
    for c in range(N_CORES):
        o = res.results[c]["out"][0]
        exp_total += float(o[0])
        for i in range(NIMG):
            inst_total += float(o[1 + i])
    int_loss = exp_total / float(B * H * W)
    inst = inst_total / float(B)
    return np.float32(int_loss - inst)



# revision 6
# speedup vs baseline: 1.0205x; 1.0205x over previous
"""Trainium2 Bass kernel for nn_PoissonNLLLoss (B=16, H=1024, W=2048, MAX_ID=356).

Computes  LOSS_WEIGHT * (mean(exp(logits)) - inst)  where inst is the mean over
images of the sum of logits gathered at per-segment centroids (segments are
label ids > 100), exactly matching the jax reference semantics.

Sharding: data-parallel over the batch — 2 images per NeuronCore across 8
cores (SPMD, identical program). Host combines the per-core partial scalars
(exp-sum and per-image instance sums), the only cross-core communication.

Per-core algorithm (all segment statistics are exact f32 integer arithmetic):
  id = 32*hi + lo. The image is processed in row-bands of 128 rows; within a
  band each column c is one "chunk" of 128 pixels (partition p = row in band).
  For each chunk one bf16 matmul accumulates into PSUM:
      psum[(s,j), i] += sum_p stat[p,(s,j)] * onehot_lo[p,i]
  with stationary stat = onehot_hi (x) {1, p, c mod 256} — all bf16-exact
  values — grouped per (band, 256-column octant). PSUM evacuations apply the
  exact f32 corrections  sy += 128*band*cnt_g  and  sx += 256*oct*cnt_g.
  One-hots are built on DVE/GPSIMD/ACT in transposed step-1 bf16 layouts
  (DVE 2x packed mode); exp+row-sum rides on ACT via accum_out.
  Finalize on device: centroids via exact floor division (reciprocal +/-1
  correction), indirect-DMA gather of logits at centroid offsets, validity
  masking, and partition reduction via a ones-matmul.
"""

import numpy as np

P = 128
NLO = 32
NHI = 12
NST = 3          # stationary stats {1, p, c mod 256}
MAX_ID = 356
NID = NLO * NHI  # 384 (ids >= 356 never occur -> cnt 0, masked)
OCT = 256        # column span of one PSUM accumulation group
NBLK = 5         # bounce blocks per image: cnt, Sp, Sc, corr_y, corr_x

B, H, W = 16, 1024, 2048
N_CORES = 8
NIMG = B // N_CORES


def _build_nc(n_img, H, W, G=128, trunc_cast=False):
    # trunc_cast: CoreSim truncates on f32->i32 copy; TRN2 HW rounds to
    # nearest. The hi-digit extraction bias must match the cast mode.
    cast_bias = 0.5 if trunc_cast else -15.5
    import concourse.bass as bass
    import concourse.bacc as bacc
    import concourse.tile as tile
    from concourse import mybir

    f32 = mybir.dt.float32
    i32 = mybir.dt.int32
    bf16 = mybir.dt.bfloat16
    Alu = mybir.AluOpType
    Act = mybir.ActivationFunctionType

    NB = H // P
    NOCT = max(W // OCT, 1)
    G = min(G, W)
    NBATCH = W // G
    BPO = max(NBATCH // NOCT, 1)
    M = NST * NHI
    n_btiles = n_img * NB

    nc = bacc.Bacc('TRN2', target_bir_lowering=False, debug=False)
    logits_h = nc.declare_dram_parameter("logits", [n_img, H, W], f32, isOutput=False)
    label_h = nc.declare_dram_parameter("label", [n_img, H, W], i32, isOutput=False)
    out_h = nc.declare_dram_parameter("out", [1, 4], f32, isOutput=True)
    bounce_h = nc.dram_tensor("bounce", [n_img * NBLK * NID], f32)

    with tile.TileContext(nc) as tc:
        import contextlib
        ctx = contextlib.ExitStack()
        with ctx:
            cpool = ctx.enter_context(tc.tile_pool(name="consts", bufs=1))
            bandA = ctx.enter_context(tc.tile_pool(name="bandA", bufs=3))
            bandB = ctx.enter_context(tc.tile_pool(name="bandB", bufs=3))
            batchp = ctx.enter_context(tc.tile_pool(name="batchp", bufs=4))
            accp = ctx.enter_context(tc.tile_pool(name="acc", bufs=1))
            psum = ctx.enter_context(tc.tile_pool(name="psum", bufs=4, space="PSUM"))
            fin = ctx.enter_context(tc.tile_pool(name="fin", bufs=1))

            # ---- constants (transposed step-1 bf16 layouts; values bf16-exact)
            iota32_t = cpool.tile([P, NLO * G], bf16)
            nc.gpsimd.iota(iota32_t[:].rearrange("p (i c) -> p i c", i=NLO),
                           pattern=[[1, NLO], [0, G]], base=0, channel_multiplier=0,
                           allow_small_or_imprecise_dtypes=True)
            iota12_t = cpool.tile([P, NHI * G], bf16)
            nc.gpsimd.iota(iota12_t[:].rearrange("p (j c) -> p j c", j=NHI),
                           pattern=[[1, NHI], [0, G]], base=0, channel_multiplier=0,
                           allow_small_or_imprecise_dtypes=True)
            OCTW = min(OCT, W)
            xr_t = cpool.tile([P, NHI * OCTW], bf16)
            nc.gpsimd.iota(xr_t[:].rearrange("p (j c) -> p j c", j=NHI),
                           pattern=[[0, NHI], [1, OCTW]], base=0, channel_multiplier=0,
                           allow_small_or_imprecise_dtypes=True)
            p_col = cpool.tile([P, 1], f32)
            nc.gpsimd.iota(p_col[:], pattern=[[0, 1]], base=0, channel_multiplier=1,
                           allow_small_or_imprecise_dtypes=True)
            ones_col = cpool.tile([P, 1], f32)
            nc.vector.memset(ones_col[:], 1.0)
            # id layout after bounce reload: id = 3*p + j at [p, img*3 + j]
            idf = cpool.tile([P, n_img * 3], f32)
            nc.gpsimd.iota(idf[:].rearrange("p (g i) -> p g i", g=n_img),
                           pattern=[[0, n_img], [1, 3]], base=0,
                           channel_multiplier=3,
                           allow_small_or_imprecise_dtypes=True)

            exp_accs = accp.tile([P, n_btiles], f32)
            octw = cpool.tile([P, NOCT], f32)
            nc.gpsimd.iota(octw[:], pattern=[[1, NOCT]], base=0,
                           channel_multiplier=0,
                           allow_small_or_imprecise_dtypes=True)
            nc.vector.tensor_scalar(out=octw[:], in0=octw[:], scalar1=float(OCT),
                                    scalar2=None, op0=Alu.mult)
            xw_scr = accp.tile([NHI, NOCT * NLO], f32)
            xw_dummy = accp.tile([NHI, 1], f32)

            accs = []
            for img in range(n_img):
                a = accp.tile([M, NLO], f32, tag=f"acc{img}")
                cy = accp.tile([NHI, NLO], f32, tag=f"accy{img}")
                cx = accp.tile([NHI, NLO], f32, tag=f"accx{img}")
                nc.vector.memset(a[:], 0.0)
                nc.vector.memset(cy[:], 0.0)
                nc.vector.memset(cx[:], 0.0)
                accs.append((a, cy, cx))

            for img in range(n_img):
                acc, acc2y, acc2x = accs[img]
                for band in range(NB):
                    r0 = band * P
                    label_band = bandA.tile([P, W], i32, tag="label_band")
                    nc.sync.dma_start(out=label_band[:], in_=label_h[img, r0:r0 + P, :])
                    logits_band = bandA.tile([P, W], f32, tag="logits_band")
                    nc.sync.dma_start(out=logits_band[:], in_=logits_h[img, r0:r0 + P, :])

                    # exp + per-partition row-sum fused on ACT
                    exp_scr = bandB.tile([P, W], bf16, tag="exp_scr")
                    nc.scalar.activation(
                        out=exp_scr[:], in_=logits_band[:], func=Act.Exp,
                        accum_out=exp_accs[:, img * NB + band: img * NB + band + 1])

                    # hi = int_cast((label + cast_bias)/32); lo = label - 32*hi
                    hi_i = bandB.tile([P, W], i32, tag="hi_i")
                    nc.vector.tensor_scalar(out=hi_i[:], in0=label_band[:],
                                            scalar1=cast_bias, scalar2=1.0 / NLO,
                                            op0=Alu.add, op1=Alu.mult)
                    hi_bf = bandB.tile([P, W], bf16, tag="hi_bf")
                    nc.scalar.activation(out=hi_bf[:], in_=hi_i[:], func=Act.Copy)
                    lo_bf = bandB.tile([P, W], bf16, tag="lo_bf")
                    nc.vector.scalar_tensor_tensor(out=lo_bf[:], in0=hi_bf[:],
                                                   scalar=-float(NLO), in1=label_band[:],
                                                   op0=Alu.mult, op1=Alu.add)

                    ps = psum.tile([M, NOCT * NLO], f32, tag="psband")
                    for oct_i in range(NOCT):
                        for bj in range(BPO):
                            bi = oct_i * BPO + bj
                            c0 = bi * G
                            # transposed layouts: innermost dim = chunk (step 1)
                            alo = batchp.tile([P, NLO * G], bf16, tag="alo")
                            alo_v = alo[:].rearrange("p (i c) -> p i c", i=NLO)
                            lo_b = lo_bf[:, c0:c0 + G].unsqueeze(1).to_broadcast([P, NLO, G])
                            nc.vector.tensor_tensor(
                                out=alo_v, in0=lo_b,
                                in1=iota32_t[:].rearrange("p (i c) -> p i c", i=NLO),
                                op=Alu.is_equal)

                            stat = batchp.tile([P, M * G], bf16, tag="stat")
                            stat_v = stat[:].rearrange("p (s j c) -> p s j c", s=NST, j=NHI)
                            hi_b = hi_bf[:, c0:c0 + G].unsqueeze(1).to_broadcast([P, NHI, G])
                            nc.vector.tensor_tensor(
                                out=stat_v[:, 0, :, :], in0=hi_b,
                                in1=iota12_t[:].rearrange("p (j c) -> p j c", j=NHI),
                                op=Alu.is_equal)
                            # stat * p on ACT (per-partition scale)
                            nc.scalar.activation(out=stat_v[:, 1, :, :],
                                                 in_=stat_v[:, 0, :, :],
                                                 func=Act.Copy, scale=p_col[:, 0:1])
                            # stat * (c mod 256) on GPSIMD
                            xr_sl = xr_t[:].rearrange("p (j c) -> p j c", j=NHI)[
                                :, :, bj * G:(bj + 1) * G]
                            nc.gpsimd.tensor_tensor(out=stat_v[:, 2, :, :],
                                                    in0=stat_v[:, 0, :, :],
                                                    in1=xr_sl, op=Alu.mult)

                            for g in range(G):
                                nc.tensor.matmul(
                                    out=ps[:, oct_i * NLO:(oct_i + 1) * NLO],
                                    lhsT=stat_v[:, :, :, g],
                                    rhs=alo_v[:, :, g],
                                    start=(bj == 0 and g == 0),
                                    stop=(bj == BPO - 1 and g == G - 1),
                                )

                    # evacuate band: band-total = sum over octants (exact f32);
                    # corr_y += 128*band*cnt_band; corr_x += 256*sum_oct oct*cnt_oct
                    ps_v = ps[:].rearrange("m (o i) -> m i o", o=NOCT)
                    bsum = bandB.tile([M, NLO], f32, tag="bsum")
                    nc.vector.tensor_reduce(out=bsum[:], in_=ps_v,
                                            axis=mybir.AxisListType.X, op=Alu.add)
                    nc.vector.tensor_tensor(out=acc[:], in0=acc[:], in1=bsum[:],
                                            op=Alu.add)
                    if band:
                        nc.vector.scalar_tensor_tensor(
                            out=acc2y[:], in0=bsum[0:NHI, :], scalar=float(P * band),
                            in1=acc2y[:], op0=Alu.mult, op1=Alu.add)
                    xw = bandB.tile([NHI, NLO], f32, tag="xw")
                    nc.vector.tensor_tensor(
                        out=xw_scr[:].rearrange("m (o i) -> m i o", o=NOCT),
                        in0=ps_v[0:NHI], in1=octw[0:NHI].unsqueeze(1).to_broadcast(
                            [NHI, NLO, NOCT]),
                        op=Alu.mult)
                    nc.vector.tensor_reduce(
                        out=xw[:], in_=xw_scr[:].rearrange("m (o i) -> m i o", o=NOCT),
                        axis=mybir.AxisListType.X, op=Alu.add)
                    nc.vector.tensor_tensor(out=acc2x[:], in0=acc2x[:], in1=xw[:],
                                            op=Alu.add)

            # ---- finalize ----
            for img in range(n_img):
                acc, acc2y, acc2x = accs[img]
                base = img * NBLK * NID
                nc.gpsimd.dma_start(
                    out=bounce_h[base:base + 3 * NID].rearrange("(p c) -> p c", p=M),
                    in_=acc[:])
                nc.gpsimd.dma_start(
                    out=bounce_h[base + 3 * NID:base + 4 * NID]
                    .rearrange("(p c) -> p c", p=NHI), in_=acc2y[:])
                nc.gpsimd.dma_start(
                    out=bounce_h[base + 4 * NID:base + 5 * NID]
                    .rearrange("(p c) -> p c", p=NHI), in_=acc2x[:])

            def reload(s):
                t = fin.tile([P, n_img * 3], f32, tag=f"re{s}")
                src = bounce_h[:].rearrange("(i s p j) -> p i s j", i=n_img, s=NBLK, p=P)
                nc.gpsimd.dma_start(out=t[:].rearrange("p (i j) -> p i j", i=n_img),
                                    in_=src[:, :, s, :])
                return t

            cnt = reload(0)
            sy = reload(1)
            sx = reload(2)
            cry = reload(3)
            crx = reload(4)
            Alu_ = Alu
            nc.vector.tensor_tensor(out=sy[:], in0=sy[:], in1=cry[:], op=Alu_.add)
            nc.vector.tensor_tensor(out=sx[:], in0=sx[:], in1=crx[:], op=Alu_.add)

            denom = fin.tile([P, n_img * 3], f32, tag="denom")
            nc.vector.tensor_scalar(out=denom[:], in0=cnt[:], scalar1=1.0, scalar2=None,
                                    op0=Alu.max)
            rcp = fin.tile([P, n_img * 3], f32, tag="rcp")
            nc.vector.reciprocal(rcp[:], denom[:])

            def floordiv(s_t, nm):
                # exact floor(s/denom): approximate quotient then +/-1 fix,
                # insensitive to the f32->i32 cast rounding mode
                qf = fin.tile([P, n_img * 3], f32, tag=f"qf{nm}")
                nc.vector.tensor_tensor(out=qf[:], in0=s_t[:], in1=rcp[:], op=Alu.mult)
                qi = fin.tile([P, n_img * 3], i32, tag=f"qi{nm}")
                nc.vector.tensor_copy(qi[:], qf[:])
                q = fin.tile([P, n_img * 3], f32, tag=f"q{nm}")
                nc.vector.tensor_copy(q[:], qi[:])
                r = fin.tile([P, n_img * 3], f32, tag=f"r{nm}")
                nc.vector.tensor_tensor(out=r[:], in0=q[:], in1=denom[:], op=Alu.mult)
                nc.vector.tensor_tensor(out=r[:], in0=s_t[:], in1=r[:], op=Alu.subtract)
                corr = fin.tile([P, n_img * 3], f32, tag=f"corr{nm}")
                nc.vector.tensor_tensor(out=corr[:], in0=r[:], in1=denom[:], op=Alu.is_ge)
                nc.vector.tensor_tensor(out=q[:], in0=q[:], in1=corr[:], op=Alu.add)
                nc.vector.tensor_scalar(out=corr[:], in0=r[:], scalar1=0.0, scalar2=None,
                                        op0=Alu.is_lt)
                nc.vector.tensor_tensor(out=q[:], in0=q[:], in1=corr[:], op=Alu.subtract)
                return q

            qy = floordiv(sy, "y")
            qx = floordiv(sx, "x")

            offs_f = fin.tile([P, n_img * 3], f32, tag="offs_f")
            nc.vector.scalar_tensor_tensor(out=offs_f[:], in0=qy[:], scalar=float(W),
                                           in1=qx[:], op0=Alu.mult, op1=Alu.add)
            mask = fin.tile([P, n_img * 3], f32, tag="mask")
            nc.vector.tensor_scalar(out=mask[:], in0=idf[:], scalar1=100.0, scalar2=None,
                                    op0=Alu.is_gt)
            m2 = fin.tile([P, n_img * 3], f32, tag="m2")
            nc.vector.tensor_scalar(out=m2[:], in0=cnt[:], scalar1=0.0, scalar2=None,
                                    op0=Alu.is_gt)
            nc.vector.tensor_tensor(out=mask[:], in0=mask[:], in1=m2[:], op=Alu.mult)
            nc.vector.tensor_tensor(out=offs_f[:], in0=offs_f[:], in1=mask[:], op=Alu.mult)
            offs_i = fin.tile([P, n_img * 3], i32, tag="offs_i")
            nc.vector.tensor_copy(offs_i[:], offs_f[:])

            # gather logits at centroids (one offset per partition per DMA)
            gath = fin.tile([P, n_img * 3], f32, tag="gath")
            for img in range(n_img):
                for j in range(3):
                    col = img * 3 + j
                    nc.gpsimd.indirect_dma_start(
                        out=gath[:, col:col + 1],
                        out_offset=None,
                        in_=logits_h[:].rearrange("i h w -> (i h w)").unsqueeze(1),
                        in_offset=bass.IndirectOffsetOnAxis(
                            ap=offs_i[:, col:col + 1], axis=0),
                        element_offset=img * H * W,
                    )

            nc.vector.tensor_tensor(out=gath[:], in0=gath[:], in1=mask[:], op=Alu.mult)

            red = fin.tile([P, n_img + 1], f32, tag="red")
            for img in range(n_img):
                nc.vector.tensor_reduce(out=red[:, img:img + 1],
                                        in_=gath[:, img * 3:(img + 1) * 3],
                                        axis=mybir.AxisListType.X, op=Alu.add)
            nc.vector.tensor_reduce(out=red[:, n_img:n_img + 1], in_=exp_accs[:],
                                    axis=mybir.AxisListType.X, op=Alu.add)

            ps_fin = psum.tile([1, n_img + 1], f32, tag="ps_fin")
            nc.tensor.matmul(out=ps_fin[:], lhsT=ones_col[:], rhs=red[:],
                             start=True, stop=True)

            out_sb = fin.tile([1, 4], f32, tag="out_sb")
            nc.vector.memset(out_sb[:], 0.0)
            nc.vector.tensor_copy(out_sb[:, 0:1], ps_fin[:, n_img:n_img + 1])
            for img in range(n_img):
                nc.vector.tensor_copy(out_sb[:, 1 + img:2 + img], ps_fin[:, img:img + 1])
            nc.gpsimd.dma_start(out=out_h[:], in_=out_sb[:])

    nc.compile()
    return nc


_NC_CACHE = {}


def kernel(logits, label):
    logits = np.ascontiguousarray(np.asarray(logits, dtype=np.float32))
    label = np.ascontiguousarray(np.asarray(label, dtype=np.int32))
    assert logits.shape == (B, H, W), logits.shape
    assert label.shape == (B, H, W), label.shape

    from concourse.bass_utils import run_bass_kernel_spmd

    key = (NIMG, H, W)
    if key not in _NC_CACHE:
        _NC_CACHE[key] = _build_nc(NIMG, H, W, G=128)
    nc = _NC_CACHE[key]

    in_maps = [
        {"logits": logits[c * NIMG:(c + 1) * NIMG],
         "label": label[c * NIMG:(c + 1) * NIMG]}
        for c in range(N_CORES)
    ]
    # the axon-proxied device occasionally reports a transient
    # NRT_EXEC_UNIT_UNRECOVERABLE; retry a few times before giving up
    import time as _time
    last_exc = None
    for attempt in range(4):
        try:
            res = run_bass_kernel_spmd(nc, in_maps, list(range(N_CORES)))
            break
        except Exception as e:  # jax.errors.JaxRuntimeError and friends
            last_exc = e
            _time.sleep(2.0 * (attempt + 1))
    else:
        raise last_exc

    # host-side combine of the per-core partial scalars (the two "all-reduces")
    exp_total = 0.0
    inst_total = 0.0
    for c in range(N_CORES):
        o = res.results[c]["out"][0]
        exp_total += float(o[0])
        for i in range(NIMG):
            inst_total += float(o[1 + i])
    int_loss = exp_total / float(B * H * W)
    inst = inst_total / float(B)
    return np.float32(int_loss - inst)



# revision 7
# speedup vs baseline: 1.0254x; 1.0049x over previous
"""Trainium2 Bass kernel for nn_PoissonNLLLoss (B=16, H=1024, W=2048, MAX_ID=356).

Computes  LOSS_WEIGHT * (mean(exp(logits)) - inst)  where inst is the mean over
images of the sum of logits gathered at per-segment centroids (segments are
label ids > 100), exactly matching the jax reference semantics.

Sharding: data-parallel over the batch — 2 images per NeuronCore across 8
cores (SPMD, identical program). Host combines the per-core partial scalars
(exp-sum and per-image instance sums), the only cross-core communication.

Per-core algorithm (all segment statistics are exact f32 integer arithmetic):
  id = 32*hi + lo. The image is processed in row-bands of 128 rows; within a
  band each column c is one "chunk" of 128 pixels (partition p = row in band).
  For each chunk one bf16 matmul accumulates into PSUM:
      psum[(s,j), i] += sum_p stat[p,(s,j)] * onehot_lo[p,i]
  with stationary stat = onehot_hi (x) {1, p, c mod 256} — all bf16-exact
  values — grouped per (band, 256-column octant). PSUM evacuations apply the
  exact f32 corrections  sy += 128*band*cnt_g  and  sx += 256*oct*cnt_g.
  One-hots are built on DVE/GPSIMD/ACT in transposed step-1 bf16 layouts
  (DVE 2x packed mode); exp+row-sum rides on ACT via accum_out.
  Finalize on device: centroids via exact floor division (reciprocal +/-1
  correction), indirect-DMA gather of logits at centroid offsets, validity
  masking, and partition reduction via a ones-matmul.
"""

import numpy as np

P = 128
NLO = 32
NHI = 12
NST = 3          # stationary stats {1, p, c mod 256}
MAX_ID = 356
NID = NLO * NHI  # 384 (ids >= 356 never occur -> cnt 0, masked)
OCT = 256        # column span of one PSUM accumulation group
NBLK = 5         # bounce blocks per image: cnt, Sp, Sc, corr_y, corr_x

B, H, W = 16, 1024, 2048
N_CORES = 8
NIMG = B // N_CORES


def _build_nc(n_img, H, W, G=128, trunc_cast=False):
    # trunc_cast: CoreSim truncates on f32->i32 copy; TRN2 HW rounds to
    # nearest. The hi-digit extraction bias must match the cast mode.
    cast_bias = 0.5 if trunc_cast else -15.5
    import concourse.bass as bass
    import concourse.bacc as bacc
    import concourse.tile as tile
    from concourse import mybir

    f32 = mybir.dt.float32
    i32 = mybir.dt.int32
    bf16 = mybir.dt.bfloat16
    Alu = mybir.AluOpType
    Act = mybir.ActivationFunctionType

    NB = H // P
    NOCT = max(W // OCT, 1)
    G = min(G, W)
    NBATCH = W // G
    BPO = max(NBATCH // NOCT, 1)
    M = NST * NHI
    n_btiles = n_img * NB

    nc = bacc.Bacc('TRN2', target_bir_lowering=False, debug=False)
    logits_h = nc.declare_dram_parameter("logits", [n_img, H, W], f32, isOutput=False)
    label_h = nc.declare_dram_parameter("label", [n_img, H, W], i32, isOutput=False)
    out_h = nc.declare_dram_parameter("out", [1, 4], f32, isOutput=True)
    bounce_h = nc.dram_tensor("bounce", [n_img * NBLK * NID], f32)

    with tile.TileContext(nc) as tc:
        import contextlib
        ctx = contextlib.ExitStack()
        with ctx:
            cpool = ctx.enter_context(tc.tile_pool(name="consts", bufs=1))
            bandA = ctx.enter_context(tc.tile_pool(name="bandA", bufs=3))
            bandB = ctx.enter_context(tc.tile_pool(name="bandB", bufs=3))
            batchp = ctx.enter_context(tc.tile_pool(name="batchp", bufs=4))
            accp = ctx.enter_context(tc.tile_pool(name="acc", bufs=1))
            psum = ctx.enter_context(tc.tile_pool(name="psum", bufs=4, space="PSUM"))
            fin = ctx.enter_context(tc.tile_pool(name="fin", bufs=1))

            # ---- constants (transposed step-1 bf16 layouts; values bf16-exact)
            iota32_t = cpool.tile([P, NLO * G], bf16)
            nc.gpsimd.iota(iota32_t[:].rearrange("p (i c) -> p i c", i=NLO),
                           pattern=[[1, NLO], [0, G]], base=0, channel_multiplier=0,
                           allow_small_or_imprecise_dtypes=True)
            iota12_t = cpool.tile([P, NHI * G], bf16)
            nc.gpsimd.iota(iota12_t[:].rearrange("p (j c) -> p j c", j=NHI),
                           pattern=[[1, NHI], [0, G]], base=0, channel_multiplier=0,
                           allow_small_or_imprecise_dtypes=True)
            OCTW = min(OCT, W)
            xr_t = cpool.tile([P, NHI * OCTW], bf16)
            nc.gpsimd.iota(xr_t[:].rearrange("p (j c) -> p j c", j=NHI),
                           pattern=[[0, NHI], [1, OCTW]], base=0, channel_multiplier=0,
                           allow_small_or_imprecise_dtypes=True)
            p_col = cpool.tile([P, 1], f32)
            nc.gpsimd.iota(p_col[:], pattern=[[0, 1]], base=0, channel_multiplier=1,
                           allow_small_or_imprecise_dtypes=True)
            ones_col = cpool.tile([P, 1], f32)
            nc.vector.memset(ones_col[:], 1.0)
            # id layout after bounce reload: id = 3*p + j at [p, img*3 + j]
            idf = cpool.tile([P, n_img * 3], f32)
            nc.gpsimd.iota(idf[:].rearrange("p (g i) -> p g i", g=n_img),
                           pattern=[[0, n_img], [1, 3]], base=0,
                           channel_multiplier=3,
                           allow_small_or_imprecise_dtypes=True)

            exp_accs = accp.tile([P, n_btiles], f32)
            octw = cpool.tile([P, NOCT], f32)
            nc.gpsimd.iota(octw[:], pattern=[[1, NOCT]], base=0,
                           channel_multiplier=0,
                           allow_small_or_imprecise_dtypes=True)
            nc.vector.tensor_scalar(out=octw[:], in0=octw[:], scalar1=float(OCT),
                                    scalar2=None, op0=Alu.mult)
            xw_scr = accp.tile([NHI, NOCT * NLO], f32)
            xw_dummy = accp.tile([NHI, 1], f32)

            accs = []
            for img in range(n_img):
                a = accp.tile([M, NLO], f32, tag=f"acc{img}")
                cy = accp.tile([NHI, NLO], f32, tag=f"accy{img}")
                cx = accp.tile([NHI, NLO], f32, tag=f"accx{img}")
                nc.vector.memset(a[:], 0.0)
                nc.vector.memset(cy[:], 0.0)
                nc.vector.memset(cx[:], 0.0)
                accs.append((a, cy, cx))

            for img in range(n_img):
                acc, acc2y, acc2x = accs[img]
                for band in range(NB):
                    r0 = band * P
                    label_band = bandA.tile([P, W], i32, tag="label_band")
                    nc.sync.dma_start(out=label_band[:], in_=label_h[img, r0:r0 + P, :])
                    logits_band = bandA.tile([P, W], f32, tag="logits_band")
                    nc.sync.dma_start(out=logits_band[:], in_=logits_h[img, r0:r0 + P, :])

                    # exp + per-partition row-sum fused on ACT
                    exp_scr = bandB.tile([P, W], bf16, tag="exp_scr")
                    nc.scalar.activation(
                        out=exp_scr[:], in_=logits_band[:], func=Act.Exp,
                        accum_out=exp_accs[:, img * NB + band: img * NB + band + 1])

                    # hi = int_cast((label + cast_bias)/32); lo = label - 32*hi
                    hi_i = bandB.tile([P, W], i32, tag="hi_i")
                    nc.vector.tensor_scalar(out=hi_i[:], in0=label_band[:],
                                            scalar1=cast_bias, scalar2=1.0 / NLO,
                                            op0=Alu.add, op1=Alu.mult)
                    hi_bf = bandB.tile([P, W], bf16, tag="hi_bf")
                    nc.scalar.activation(out=hi_bf[:], in_=hi_i[:], func=Act.Copy)
                    lo_bf = bandB.tile([P, W], bf16, tag="lo_bf")
                    nc.vector.scalar_tensor_tensor(out=lo_bf[:], in0=hi_bf[:],
                                                   scalar=-float(NLO), in1=label_band[:],
                                                   op0=Alu.mult, op1=Alu.add)

                    ps = psum.tile([M, NOCT * NLO], f32, tag="psband")
                    for oct_i in range(NOCT):
                        for bj in range(BPO):
                            bi = oct_i * BPO + bj
                            c0 = bi * G
                            # transposed layouts: innermost dim = chunk (step 1)
                            alo = batchp.tile([P, NLO * G], bf16, tag="alo")
                            alo_v = alo[:].rearrange("p (i c) -> p i c", i=NLO)
                            lo_b = lo_bf[:, c0:c0 + G].unsqueeze(1).to_broadcast([P, NLO, G])
                            nc.vector.tensor_tensor(
                                out=alo_v, in0=lo_b,
                                in1=iota32_t[:].rearrange("p (i c) -> p i c", i=NLO),
                                op=Alu.is_equal)

                            stat = batchp.tile([P, M * G], bf16, tag="stat")
                            stat_v = stat[:].rearrange("p (s j c) -> p s j c", s=NST, j=NHI)
                            hi_b = hi_bf[:, c0:c0 + G].unsqueeze(1).to_broadcast([P, NHI, G])
                            nc.vector.tensor_tensor(
                                out=stat_v[:, 0, :, :], in0=hi_b,
                                in1=iota12_t[:].rearrange("p (j c) -> p j c", j=NHI),
                                op=Alu.is_equal)
                            # stat * p on ACT (per-partition scale)
                            nc.scalar.activation(out=stat_v[:, 1, :, :],
                                                 in_=stat_v[:, 0, :, :],
                                                 func=Act.Copy, scale=p_col[:, 0:1])
                            # stat * (c mod 256) on GPSIMD
                            xr_sl = xr_t[:].rearrange("p (j c) -> p j c", j=NHI)[
                                :, :, bj * G:(bj + 1) * G]
                            nc.gpsimd.tensor_tensor(out=stat_v[:, 2, :, :],
                                                    in0=stat_v[:, 0, :, :],
                                                    in1=xr_sl, op=Alu.mult)

                            for g in range(G):
                                nc.tensor.matmul(
                                    out=ps[:, oct_i * NLO:(oct_i + 1) * NLO],
                                    lhsT=stat_v[:, :, :, g],
                                    rhs=alo_v[:, :, g],
                                    start=(bj == 0 and g == 0),
                                    stop=(bj == BPO - 1 and g == G - 1),
                                )

                    # evacuate band: band-total = sum over octants (exact f32);
                    # corr_y += 128*band*cnt_band; corr_x += 256*sum_oct oct*cnt_oct
                    ps_v = ps[:].rearrange("m (o i) -> m i o", o=NOCT)
                    bsum = bandB.tile([M, NLO], f32, tag="bsum")
                    nc.vector.tensor_reduce(out=bsum[:], in_=ps_v,
                                            axis=mybir.AxisListType.X, op=Alu.add)
                    nc.vector.tensor_tensor(out=acc[:], in0=acc[:], in1=bsum[:],
                                            op=Alu.add)
                    if band:
                        nc.vector.scalar_tensor_tensor(
                            out=acc2y[:], in0=bsum[0:NHI, :], scalar=float(P * band),
                            in1=acc2y[:], op0=Alu.mult, op1=Alu.add)
                    xw = bandB.tile([NHI, NLO], f32, tag="xw")
                    nc.vector.tensor_tensor(
                        out=xw_scr[:].rearrange("m (o i) -> m i o", o=NOCT),
                        in0=ps_v[0:NHI], in1=octw[0:NHI].unsqueeze(1).to_broadcast(
                            [NHI, NLO, NOCT]),
                        op=Alu.mult)
                    nc.vector.tensor_reduce(
                        out=xw[:], in_=xw_scr[:].rearrange("m (o i) -> m i o", o=NOCT),
                        axis=mybir.AxisListType.X, op=Alu.add)
                    nc.vector.tensor_tensor(out=acc2x[:], in0=acc2x[:], in1=xw[:],
                                            op=Alu.add)

            # ---- finalize ----
            for img in range(n_img):
                acc, acc2y, acc2x = accs[img]
                base = img * NBLK * NID
                nc.sync.dma_start(
                    out=bounce_h[base:base + 3 * NID].rearrange("(p c) -> p c", p=M),
                    in_=acc[:])
                nc.sync.dma_start(
                    out=bounce_h[base + 3 * NID:base + 4 * NID]
                    .rearrange("(p c) -> p c", p=NHI), in_=acc2y[:])
                nc.sync.dma_start(
                    out=bounce_h[base + 4 * NID:base + 5 * NID]
                    .rearrange("(p c) -> p c", p=NHI), in_=acc2x[:])

            def reload(s):
                t = fin.tile([P, n_img * 3], f32, tag=f"re{s}")
                src = bounce_h[:].rearrange("(i s p j) -> p i s j", i=n_img, s=NBLK, p=P)
                nc.sync.dma_start(out=t[:].rearrange("p (i j) -> p i j", i=n_img),
                                    in_=src[:, :, s, :])
                return t

            cnt = reload(0)
            sy = reload(1)
            sx = reload(2)
            cry = reload(3)
            crx = reload(4)
            Alu_ = Alu
            nc.vector.tensor_tensor(out=sy[:], in0=sy[:], in1=cry[:], op=Alu_.add)
            nc.vector.tensor_tensor(out=sx[:], in0=sx[:], in1=crx[:], op=Alu_.add)

            denom = fin.tile([P, n_img * 3], f32, tag="denom")
            nc.vector.tensor_scalar(out=denom[:], in0=cnt[:], scalar1=1.0, scalar2=None,
                                    op0=Alu.max)
            rcp = fin.tile([P, n_img * 3], f32, tag="rcp")
            nc.vector.reciprocal(rcp[:], denom[:])

            def floordiv(s_t, nm):
                # exact floor(s/denom): approximate quotient then +/-1 fix,
                # insensitive to the f32->i32 cast rounding mode
                qf = fin.tile([P, n_img * 3], f32, tag=f"qf{nm}")
                nc.vector.tensor_tensor(out=qf[:], in0=s_t[:], in1=rcp[:], op=Alu.mult)
                qi = fin.tile([P, n_img * 3], i32, tag=f"qi{nm}")
                nc.vector.tensor_copy(qi[:], qf[:])
                q = fin.tile([P, n_img * 3], f32, tag=f"q{nm}")
                nc.vector.tensor_copy(q[:], qi[:])
                r = fin.tile([P, n_img * 3], f32, tag=f"r{nm}")
                nc.vector.tensor_tensor(out=r[:], in0=q[:], in1=denom[:], op=Alu.mult)
                nc.vector.tensor_tensor(out=r[:], in0=s_t[:], in1=r[:], op=Alu.subtract)
                corr = fin.tile([P, n_img * 3], f32, tag=f"corr{nm}")
                nc.vector.tensor_tensor(out=corr[:], in0=r[:], in1=denom[:], op=Alu.is_ge)
                nc.vector.tensor_tensor(out=q[:], in0=q[:], in1=corr[:], op=Alu.add)
                nc.vector.tensor_scalar(out=corr[:], in0=r[:], scalar1=0.0, scalar2=None,
                                        op0=Alu.is_lt)
                nc.vector.tensor_tensor(out=q[:], in0=q[:], in1=corr[:], op=Alu.subtract)
                return q

            qy = floordiv(sy, "y")
            qx = floordiv(sx, "x")

            offs_f = fin.tile([P, n_img * 3], f32, tag="offs_f")
            nc.vector.scalar_tensor_tensor(out=offs_f[:], in0=qy[:], scalar=float(W),
                                           in1=qx[:], op0=Alu.mult, op1=Alu.add)
            mask = fin.tile([P, n_img * 3], f32, tag="mask")
            nc.vector.tensor_scalar(out=mask[:], in0=idf[:], scalar1=100.0, scalar2=None,
                                    op0=Alu.is_gt)
            m2 = fin.tile([P, n_img * 3], f32, tag="m2")
            nc.vector.tensor_scalar(out=m2[:], in0=cnt[:], scalar1=0.0, scalar2=None,
                                    op0=Alu.is_gt)
            nc.vector.tensor_tensor(out=mask[:], in0=mask[:], in1=m2[:], op=Alu.mult)
            nc.vector.tensor_tensor(out=offs_f[:], in0=offs_f[:], in1=mask[:], op=Alu.mult)
            offs_i = fin.tile([P, n_img * 3], i32, tag="offs_i")
            nc.vector.tensor_copy(offs_i[:], offs_f[:])

            # gather logits at centroids (one offset per partition per DMA)
            gath = fin.tile([P, n_img * 3], f32, tag="gath")
            for img in range(n_img):
                for j in range(3):
                    col = img * 3 + j
                    nc.gpsimd.indirect_dma_start(
                        out=gath[:, col:col + 1],
                        out_offset=None,
                        in_=logits_h[:].rearrange("i h w -> (i h w)").unsqueeze(1),
                        in_offset=bass.IndirectOffsetOnAxis(
                            ap=offs_i[:, col:col + 1], axis=0),
                        element_offset=img * H * W,
                    )

            nc.vector.tensor_tensor(out=gath[:], in0=gath[:], in1=mask[:], op=Alu.mult)

            red = fin.tile([P, n_img + 1], f32, tag="red")
            for img in range(n_img):
                nc.vector.tensor_reduce(out=red[:, img:img + 1],
                                        in_=gath[:, img * 3:(img + 1) * 3],
                                        axis=mybir.AxisListType.X, op=Alu.add)
            nc.vector.tensor_reduce(out=red[:, n_img:n_img + 1], in_=exp_accs[:],
                                    axis=mybir.AxisListType.X, op=Alu.add)

            ps_fin = psum.tile([1, n_img + 1], f32, tag="ps_fin")
            nc.tensor.matmul(out=ps_fin[:], lhsT=ones_col[:], rhs=red[:],
                             start=True, stop=True)

            out_sb = fin.tile([1, 4], f32, tag="out_sb")
            nc.vector.memset(out_sb[:], 0.0)
            nc.vector.tensor_copy(out_sb[:, 0:1], ps_fin[:, n_img:n_img + 1])
            for img in range(n_img):
                nc.vector.tensor_copy(out_sb[:, 1 + img:2 + img], ps_fin[:, img:img + 1])
            nc.sync.dma_start(out=out_h[:], in_=out_sb[:])

    nc.compile()
    return nc


_NC_CACHE = {}


def kernel(logits, label):
    logits = np.ascontiguousarray(np.asarray(logits, dtype=np.float32))
    label = np.ascontiguousarray(np.asarray(label, dtype=np.int32))
    assert logits.shape == (B, H, W), logits.shape
    assert label.shape == (B, H, W), label.shape

    from concourse.bass_utils import run_bass_kernel_spmd

    key = (NIMG, H, W)
    if key not in _NC_CACHE:
        _NC_CACHE[key] = _build_nc(NIMG, H, W, G=128)
    nc = _NC_CACHE[key]

    in_maps = [
        {"logits": logits[c * NIMG:(c + 1) * NIMG],
         "label": label[c * NIMG:(c + 1) * NIMG]}
        for c in range(N_CORES)
    ]
    # the axon-proxied device occasionally reports a transient
    # NRT_EXEC_UNIT_UNRECOVERABLE; retry a few times before giving up
    import time as _time
    last_exc = None
    for attempt in range(4):
        try:
            res = run_bass_kernel_spmd(nc, in_maps, list(range(N_CORES)))
            break
        except Exception as e:  # jax.errors.JaxRuntimeError and friends
            last_exc = e
            _time.sleep(2.0 * (attempt + 1))
    else:
        raise last_exc

    # host-side combine of the per-core partial scalars (the two "all-reduces")
    exp_total = 0.0
    inst_total = 0.0
    for c in range(N_CORES):
        o = res.results[c]["out"][0]
        exp_total += float(o[0])
        for i in range(NIMG):
            inst_total += float(o[1 + i])
    int_loss = exp_total / float(B * H * W)
    inst = inst_total / float(B)
    return np.float32(int_loss - inst)



# revision 8
# speedup vs baseline: 1.0560x; 1.0298x over previous
"""Trainium2 Bass kernel for nn_PoissonNLLLoss (B=16, H=1024, W=2048, MAX_ID=356).

Computes  LOSS_WEIGHT * (mean(exp(logits)) - inst)  where inst is the mean over
images of the sum of logits gathered at per-segment centroids (segments are
label ids > 100), exactly matching the jax reference semantics.

Sharding: data-parallel over the batch — 2 images per NeuronCore across 8
cores (SPMD, identical program). Host combines the per-core partial scalars
(exp-sum and per-image instance sums), the only cross-core communication.

Per-core algorithm (all segment statistics are exact f32 integer arithmetic):
  id = 32*hi + lo. The image is processed in row-bands of 128 rows; within a
  band each column c is one "chunk" of 128 pixels (partition p = row in band).
  For each chunk one bf16 matmul accumulates into PSUM:
      psum[(s,j), i] += sum_p stat[p,(s,j)] * onehot_lo[p,i]
  with stationary stat = onehot_hi (x) {1, p, c mod 256} — all bf16-exact
  values — grouped per (band, 256-column octant). PSUM evacuations apply the
  exact f32 corrections  sy += 128*band*cnt_g  and  sx += 256*oct*cnt_g.
  One-hots are built on DVE/GPSIMD/ACT in transposed step-1 bf16 layouts
  (DVE 2x packed mode); exp+row-sum rides on ACT via accum_out.
  Finalize on device: centroids via exact floor division (reciprocal +/-1
  correction), indirect-DMA gather of logits at centroid offsets, validity
  masking, and partition reduction via a ones-matmul.
"""

import numpy as np

P = 128
NLO = 32
NHI = 12
NST = 3          # stationary stats {1, p, c mod 256}
MAX_ID = 356
NID = NLO * NHI  # 384 (ids >= 356 never occur -> cnt 0, masked)
OCT = 256        # column span of one PSUM accumulation group
NBLK = 5         # bounce blocks per image: cnt, Sp, Sc, corr_y, corr_x

B, H, W = 16, 1024, 2048
N_CORES = 8
NIMG = B // N_CORES


def _build_nc(n_img, H, W, G=128, trunc_cast=False):
    # trunc_cast: CoreSim truncates on f32->i32 copy; TRN2 HW rounds to
    # nearest. The hi-digit extraction bias must match the cast mode.
    cast_bias = 0.5 if trunc_cast else -15.5
    import concourse.bass as bass
    import concourse.bacc as bacc
    import concourse.tile as tile
    from concourse import mybir

    f32 = mybir.dt.float32
    i32 = mybir.dt.int32
    bf16 = mybir.dt.bfloat16
    Alu = mybir.AluOpType
    Act = mybir.ActivationFunctionType

    NB = H // P
    NOCT = max(W // OCT, 1)
    G = min(G, W)
    NBATCH = W // G
    BPO = max(NBATCH // NOCT, 1)
    M = NST * NHI
    n_btiles = n_img * NB

    nc = bacc.Bacc('TRN2', target_bir_lowering=False, debug=False)
    logits_h = nc.declare_dram_parameter("logits", [n_img, H, W], f32, isOutput=False)
    label_h = nc.declare_dram_parameter("label", [n_img, H, W], i32, isOutput=False)
    out_h = nc.declare_dram_parameter("out", [1, 4], f32, isOutput=True)
    bounce_h = nc.dram_tensor("bounce", [n_img * NBLK * NID], f32)

    with tile.TileContext(nc) as tc:
        import contextlib
        ctx = contextlib.ExitStack()
        with ctx:
            cpool = ctx.enter_context(tc.tile_pool(name="consts", bufs=1))
            bandA = ctx.enter_context(tc.tile_pool(name="bandA", bufs=3))
            bandB = ctx.enter_context(tc.tile_pool(name="bandB", bufs=3))
            batchp = ctx.enter_context(tc.tile_pool(name="batchp", bufs=4))
            accp = ctx.enter_context(tc.tile_pool(name="acc", bufs=1))
            psum = ctx.enter_context(tc.tile_pool(name="psum", bufs=4, space="PSUM"))
            fin = ctx.enter_context(tc.tile_pool(name="fin", bufs=1))

            # ---- constants (transposed step-1 bf16 layouts; values bf16-exact)
            iota32_t = cpool.tile([P, NLO * G], bf16)
            nc.gpsimd.iota(iota32_t[:].rearrange("p (i c) -> p i c", i=NLO),
                           pattern=[[1, NLO], [0, G]], base=0, channel_multiplier=0,
                           allow_small_or_imprecise_dtypes=True)
            iota12_t = cpool.tile([P, NHI * G], bf16)
            nc.gpsimd.iota(iota12_t[:].rearrange("p (j c) -> p j c", j=NHI),
                           pattern=[[1, NHI], [0, G]], base=0, channel_multiplier=0,
                           allow_small_or_imprecise_dtypes=True)
            OCTW = min(OCT, W)
            xr_t = cpool.tile([P, NHI * OCTW], bf16)
            nc.gpsimd.iota(xr_t[:].rearrange("p (j c) -> p j c", j=NHI),
                           pattern=[[0, NHI], [1, OCTW]], base=0, channel_multiplier=0,
                           allow_small_or_imprecise_dtypes=True)
            p_col = cpool.tile([P, 1], f32)
            nc.gpsimd.iota(p_col[:], pattern=[[0, 1]], base=0, channel_multiplier=1,
                           allow_small_or_imprecise_dtypes=True)
            ones_col = cpool.tile([P, 1], f32)
            nc.vector.memset(ones_col[:], 1.0)
            hsc_col = cpool.tile([P, 1], f32)
            nc.vector.memset(hsc_col[:], 1.0 / NLO)
            hbi_col = cpool.tile([P, 1], f32)
            nc.vector.memset(hbi_col[:], cast_bias / NLO)
            # id layout after bounce reload: id = 3*p + j at [p, img*3 + j]
            idf = cpool.tile([P, n_img * 3], f32)
            nc.gpsimd.iota(idf[:].rearrange("p (g i) -> p g i", g=n_img),
                           pattern=[[0, n_img], [1, 3]], base=0,
                           channel_multiplier=3,
                           allow_small_or_imprecise_dtypes=True)

            exp_accs = accp.tile([P, n_btiles], f32)
            octw = cpool.tile([P, NOCT], f32)
            nc.gpsimd.iota(octw[:], pattern=[[1, NOCT]], base=0,
                           channel_multiplier=0,
                           allow_small_or_imprecise_dtypes=True)
            nc.vector.tensor_scalar(out=octw[:], in0=octw[:], scalar1=float(OCT),
                                    scalar2=None, op0=Alu.mult)
            xw_scr = accp.tile([NHI, NOCT * NLO], f32)
            xw_dummy = accp.tile([NHI, 1], f32)

            accs = []
            for img in range(n_img):
                a = accp.tile([M, NLO], f32, tag=f"acc{img}")
                cy = accp.tile([NHI, NLO], f32, tag=f"accy{img}")
                cx = accp.tile([NHI, NLO], f32, tag=f"accx{img}")
                nc.vector.memset(a[:], 0.0)
                nc.vector.memset(cy[:], 0.0)
                nc.vector.memset(cx[:], 0.0)
                accs.append((a, cy, cx))

            for img in range(n_img):
                acc, acc2y, acc2x = accs[img]
                for band in range(NB):
                    r0 = band * P
                    label_band = bandA.tile([P, W], i32, tag="label_band")
                    nc.sync.dma_start(out=label_band[:], in_=label_h[img, r0:r0 + P, :])
                    logits_band = bandA.tile([P, W], f32, tag="logits_band")
                    nc.sync.dma_start(out=logits_band[:], in_=logits_h[img, r0:r0 + P, :])

                    # exp + per-partition row-sum fused on ACT
                    exp_scr = bandB.tile([P, W], bf16, tag="exp_scr")
                    nc.scalar.activation(
                        out=exp_scr[:], in_=logits_band[:], func=Act.Exp,
                        accum_out=exp_accs[:, img * NB + band: img * NB + band + 1])

                    # hi = int_cast((label + cast_bias)/32); lo = label - 32*hi
                    hi_i = bandB.tile([P, W], i32, tag="hi_i")
                    nc.scalar.activation(out=hi_i[:], in_=label_band[:],
                                         func=Act.Relu, scale=hsc_col[:, 0:1],
                                         bias=hbi_col[:, 0:1])
                    hi_bf = bandB.tile([P, W], bf16, tag="hi_bf")
                    nc.scalar.activation(out=hi_bf[:], in_=hi_i[:], func=Act.Copy)
                    lo_bf = bandB.tile([P, W], bf16, tag="lo_bf")
                    nc.vector.scalar_tensor_tensor(out=lo_bf[:], in0=hi_bf[:],
                                                   scalar=-float(NLO), in1=label_band[:],
                                                   op0=Alu.mult, op1=Alu.add)

                    ps = psum.tile([M, NOCT * NLO], f32, tag="psband")
                    for oct_i in range(NOCT):
                        for bj in range(BPO):
                            bi = oct_i * BPO + bj
                            c0 = bi * G
                            # transposed layouts: innermost dim = chunk (step 1)
                            alo = batchp.tile([P, NLO * G], bf16, tag="alo")
                            alo_v = alo[:].rearrange("p (i c) -> p i c", i=NLO)
                            lo_b = lo_bf[:, c0:c0 + G].unsqueeze(1).to_broadcast([P, NLO, G])
                            nc.vector.tensor_tensor(
                                out=alo_v, in0=lo_b,
                                in1=iota32_t[:].rearrange("p (i c) -> p i c", i=NLO),
                                op=Alu.is_equal)

                            stat = batchp.tile([P, M * G], bf16, tag="stat")
                            stat_v = stat[:].rearrange("p (s j c) -> p s j c", s=NST, j=NHI)
                            hi_b = hi_bf[:, c0:c0 + G].unsqueeze(1).to_broadcast([P, NHI, G])
                            nc.vector.tensor_tensor(
                                out=stat_v[:, 0, :, :], in0=hi_b,
                                in1=iota12_t[:].rearrange("p (j c) -> p j c", j=NHI),
                                op=Alu.is_equal)
                            # stat * p on ACT (per-partition scale)
                            nc.scalar.activation(out=stat_v[:, 1, :, :],
                                                 in_=stat_v[:, 0, :, :],
                                                 func=Act.Copy, scale=p_col[:, 0:1])
                            # stat * (c mod 256) on GPSIMD
                            xr_sl = xr_t[:].rearrange("p (j c) -> p j c", j=NHI)[
                                :, :, bj * G:(bj + 1) * G]
                            nc.gpsimd.tensor_tensor(out=stat_v[:, 2, :, :],
                                                    in0=stat_v[:, 0, :, :],
                                                    in1=xr_sl, op=Alu.mult)

                            for g in range(G):
                                nc.tensor.matmul(
                                    out=ps[:, oct_i * NLO:(oct_i + 1) * NLO],
                                    lhsT=stat_v[:, :, :, g],
                                    rhs=alo_v[:, :, g],
                                    start=(bj == 0 and g == 0),
                                    stop=(bj == BPO - 1 and g == G - 1),
                                )

                    # evacuate band: band-total = sum over octants (exact f32);
                    # corr_y += 128*band*cnt_band; corr_x += 256*sum_oct oct*cnt_oct
                    ps_v = ps[:].rearrange("m (o i) -> m i o", o=NOCT)
                    bsum = bandB.tile([M, NLO], f32, tag="bsum")
                    nc.vector.tensor_reduce(out=bsum[:], in_=ps_v,
                                            axis=mybir.AxisListType.X, op=Alu.add)
                    nc.vector.tensor_tensor(out=acc[:], in0=acc[:], in1=bsum[:],
                                            op=Alu.add)
                    if band:
                        nc.vector.scalar_tensor_tensor(
                            out=acc2y[:], in0=bsum[0:NHI, :], scalar=float(P * band),
                            in1=acc2y[:], op0=Alu.mult, op1=Alu.add)
                    xw = bandB.tile([NHI, NLO], f32, tag="xw")
                    nc.vector.tensor_tensor(
                        out=xw_scr[:].rearrange("m (o i) -> m i o", o=NOCT),
                        in0=ps_v[0:NHI], in1=octw[0:NHI].unsqueeze(1).to_broadcast(
                            [NHI, NLO, NOCT]),
                        op=Alu.mult)
                    nc.vector.tensor_reduce(
                        out=xw[:], in_=xw_scr[:].rearrange("m (o i) -> m i o", o=NOCT),
                        axis=mybir.AxisListType.X, op=Alu.add)
                    nc.vector.tensor_tensor(out=acc2x[:], in0=acc2x[:], in1=xw[:],
                                            op=Alu.add)

            # ---- finalize ----
            for img in range(n_img):
                acc, acc2y, acc2x = accs[img]
                base = img * NBLK * NID
                nc.sync.dma_start(
                    out=bounce_h[base:base + 3 * NID].rearrange("(p c) -> p c", p=M),
                    in_=acc[:])
                nc.sync.dma_start(
                    out=bounce_h[base + 3 * NID:base + 4 * NID]
                    .rearrange("(p c) -> p c", p=NHI), in_=acc2y[:])
                nc.sync.dma_start(
                    out=bounce_h[base + 4 * NID:base + 5 * NID]
                    .rearrange("(p c) -> p c", p=NHI), in_=acc2x[:])

            def reload(s):
                t = fin.tile([P, n_img * 3], f32, tag=f"re{s}")
                src = bounce_h[:].rearrange("(i s p j) -> p i s j", i=n_img, s=NBLK, p=P)
                nc.sync.dma_start(out=t[:].rearrange("p (i j) -> p i j", i=n_img),
                                    in_=src[:, :, s, :])
                return t

            cnt = reload(0)
            sy = reload(1)
            sx = reload(2)
            cry = reload(3)
            crx = reload(4)
            Alu_ = Alu
            nc.vector.tensor_tensor(out=sy[:], in0=sy[:], in1=cry[:], op=Alu_.add)
            nc.vector.tensor_tensor(out=sx[:], in0=sx[:], in1=crx[:], op=Alu_.add)

            denom = fin.tile([P, n_img * 3], f32, tag="denom")
            nc.vector.tensor_scalar(out=denom[:], in0=cnt[:], scalar1=1.0, scalar2=None,
                                    op0=Alu.max)
            rcp = fin.tile([P, n_img * 3], f32, tag="rcp")
            nc.vector.reciprocal(rcp[:], denom[:])

            def floordiv(s_t, nm):
                # exact floor(s/denom): approximate quotient then +/-1 fix,
                # insensitive to the f32->i32 cast rounding mode
                qf = fin.tile([P, n_img * 3], f32, tag=f"qf{nm}")
                nc.vector.tensor_tensor(out=qf[:], in0=s_t[:], in1=rcp[:], op=Alu.mult)
                qi = fin.tile([P, n_img * 3], i32, tag=f"qi{nm}")
                nc.vector.tensor_copy(qi[:], qf[:])
                q = fin.tile([P, n_img * 3], f32, tag=f"q{nm}")
                nc.vector.tensor_copy(q[:], qi[:])
                r = fin.tile([P, n_img * 3], f32, tag=f"r{nm}")
                nc.vector.tensor_tensor(out=r[:], in0=q[:], in1=denom[:], op=Alu.mult)
                nc.vector.tensor_tensor(out=r[:], in0=s_t[:], in1=r[:], op=Alu.subtract)
                corr = fin.tile([P, n_img * 3], f32, tag=f"corr{nm}")
                nc.vector.tensor_tensor(out=corr[:], in0=r[:], in1=denom[:], op=Alu.is_ge)
                nc.vector.tensor_tensor(out=q[:], in0=q[:], in1=corr[:], op=Alu.add)
                nc.vector.tensor_scalar(out=corr[:], in0=r[:], scalar1=0.0, scalar2=None,
                                        op0=Alu.is_lt)
                nc.vector.tensor_tensor(out=q[:], in0=q[:], in1=corr[:], op=Alu.subtract)
                return q

            qy = floordiv(sy, "y")
            qx = floordiv(sx, "x")

            offs_f = fin.tile([P, n_img * 3], f32, tag="offs_f")
            nc.vector.scalar_tensor_tensor(out=offs_f[:], in0=qy[:], scalar=float(W),
                                           in1=qx[:], op0=Alu.mult, op1=Alu.add)
            mask = fin.tile([P, n_img * 3], f32, tag="mask")
            nc.vector.tensor_scalar(out=mask[:], in0=idf[:], scalar1=100.0, scalar2=None,
                                    op0=Alu.is_gt)
            m2 = fin.tile([P, n_img * 3], f32, tag="m2")
            nc.vector.tensor_scalar(out=m2[:], in0=cnt[:], scalar1=0.0, scalar2=None,
                                    op0=Alu.is_gt)
            nc.vector.tensor_tensor(out=mask[:], in0=mask[:], in1=m2[:], op=Alu.mult)
            nc.vector.tensor_tensor(out=offs_f[:], in0=offs_f[:], in1=mask[:], op=Alu.mult)
            offs_i = fin.tile([P, n_img * 3], i32, tag="offs_i")
            nc.vector.tensor_copy(offs_i[:], offs_f[:])

            # gather logits at centroids (one offset per partition per DMA)
            gath = fin.tile([P, n_img * 3], f32, tag="gath")
            for img in range(n_img):
                for j in range(3):
                    col = img * 3 + j
                    nc.gpsimd.indirect_dma_start(
                        out=gath[:, col:col + 1],
                        out_offset=None,
                        in_=logits_h[:].rearrange("i h w -> (i h w)").unsqueeze(1),
                        in_offset=bass.IndirectOffsetOnAxis(
                            ap=offs_i[:, col:col + 1], axis=0),
                        element_offset=img * H * W,
                    )

            nc.vector.tensor_tensor(out=gath[:], in0=gath[:], in1=mask[:], op=Alu.mult)

            red = fin.tile([P, n_img + 1], f32, tag="red")
            for img in range(n_img):
                nc.vector.tensor_reduce(out=red[:, img:img + 1],
                                        in_=gath[:, img * 3:(img + 1) * 3],
                                        axis=mybir.AxisListType.X, op=Alu.add)
            nc.vector.tensor_reduce(out=red[:, n_img:n_img + 1], in_=exp_accs[:],
                                    axis=mybir.AxisListType.X, op=Alu.add)

            ps_fin = psum.tile([1, n_img + 1], f32, tag="ps_fin")
            nc.tensor.matmul(out=ps_fin[:], lhsT=ones_col[:], rhs=red[:],
                             start=True, stop=True)

            out_sb = fin.tile([1, 4], f32, tag="out_sb")
            nc.vector.memset(out_sb[:], 0.0)
            nc.vector.tensor_copy(out_sb[:, 0:1], ps_fin[:, n_img:n_img + 1])
            for img in range(n_img):
                nc.vector.tensor_copy(out_sb[:, 1 + img:2 + img], ps_fin[:, img:img + 1])
            nc.sync.dma_start(out=out_h[:], in_=out_sb[:])

    nc.compile()
    return nc


_NC_CACHE = {}


def kernel(logits, label):
    logits = np.ascontiguousarray(np.asarray(logits, dtype=np.float32))
    label = np.ascontiguousarray(np.asarray(label, dtype=np.int32))
    assert logits.shape == (B, H, W), logits.shape
    assert label.shape == (B, H, W), label.shape

    from concourse.bass_utils import run_bass_kernel_spmd

    key = (NIMG, H, W)
    if key not in _NC_CACHE:
        _NC_CACHE[key] = _build_nc(NIMG, H, W, G=128)
    nc = _NC_CACHE[key]

    in_maps = [
        {"logits": logits[c * NIMG:(c + 1) * NIMG],
         "label": label[c * NIMG:(c + 1) * NIMG]}
        for c in range(N_CORES)
    ]
    # the axon-proxied device occasionally reports a transient
    # NRT_EXEC_UNIT_UNRECOVERABLE; retry a few times before giving up
    import time as _time
    last_exc = None
    for attempt in range(4):
        try:
            res = run_bass_kernel_spmd(nc, in_maps, list(range(N_CORES)))
            break
        except Exception as e:  # jax.errors.JaxRuntimeError and friends
            last_exc = e
            _time.sleep(2.0 * (attempt + 1))
    else:
        raise last_exc

    # host-side combine of the per-core partial scalars (the two "all-reduces")
    exp_total = 0.0
    inst_total = 0.0
    for c in range(N_CORES):
        o = res.results[c]["out"][0]
        exp_total += float(o[0])
        for i in range(NIMG):
            inst_total += float(o[1 + i])
    int_loss = exp_total / float(B * H * W)
    inst = inst_total / float(B)
    return np.float32(int_loss - inst)



# revision 9
# speedup vs baseline: 1.0596x; 1.0034x over previous
"""Trainium2 Bass kernel for nn_PoissonNLLLoss (B=16, H=1024, W=2048, MAX_ID=356).

Computes  LOSS_WEIGHT * (mean(exp(logits)) - inst)  where inst is the mean over
images of the sum of logits gathered at per-segment centroids (segments are
label ids > 100), exactly matching the jax reference semantics.

Sharding: data-parallel over the batch — 2 images per NeuronCore across 8
cores (SPMD, identical program). Host combines the per-core partial scalars
(exp-sum and per-image instance sums), the only cross-core communication.

Per-core algorithm (all segment statistics are exact f32 integer arithmetic):
  id = 32*hi + lo. The image is processed in row-bands of 128 rows; within a
  band each column c is one "chunk" of 128 pixels (partition p = row in band).
  For each chunk one bf16 matmul accumulates into PSUM:
      psum[(s,j), i] += sum_p stat[p,(s,j)] * onehot_lo[p,i]
  with stationary stat = onehot_hi (x) {1, p, c mod 256} — all bf16-exact
  values — grouped per (band, 256-column octant). PSUM evacuations apply the
  exact f32 corrections  sy += 128*band*cnt_g  and  sx += 256*oct*cnt_g.
  One-hots are built on DVE/GPSIMD/ACT in transposed step-1 bf16 layouts
  (DVE 2x packed mode); exp+row-sum rides on ACT via accum_out.
  Finalize on device: centroids via exact floor division (reciprocal +/-1
  correction), indirect-DMA gather of logits at centroid offsets, validity
  masking, and partition reduction via a ones-matmul.
"""

import numpy as np

P = 128
NLO = 32
NHI = 12
NST = 3          # stationary stats {1, p, c mod 256}
MAX_ID = 356
NID = NLO * NHI  # 384 (ids >= 356 never occur -> cnt 0, masked)
OCT = 256        # column span of one PSUM accumulation group
NBLK = 5         # bounce blocks per image: cnt, Sp, Sc, corr_y, corr_x

B, H, W = 16, 1024, 2048
N_CORES = 8
NIMG = B // N_CORES


def _build_nc(n_img, H, W, G=128, trunc_cast=False):
    # trunc_cast: CoreSim truncates on f32->i32 copy; TRN2 HW rounds to
    # nearest. The hi-digit extraction bias must match the cast mode.
    cast_bias = 0.5 if trunc_cast else -15.5
    import concourse.bass as bass
    import concourse.bacc as bacc
    import concourse.tile as tile
    from concourse import mybir

    f32 = mybir.dt.float32
    i32 = mybir.dt.int32
    bf16 = mybir.dt.bfloat16
    Alu = mybir.AluOpType
    Act = mybir.ActivationFunctionType

    NB = H // P
    NOCT = max(W // OCT, 1)
    G = min(G, W)
    NBATCH = W // G
    BPO = max(NBATCH // NOCT, 1)
    M = NST * NHI
    n_btiles = n_img * NB

    nc = bacc.Bacc('TRN2', target_bir_lowering=False, debug=False)
    logits_h = nc.declare_dram_parameter("logits", [n_img, H, W], f32, isOutput=False)
    label_h = nc.declare_dram_parameter("label", [n_img, H, W], i32, isOutput=False)
    out_h = nc.declare_dram_parameter("out", [1, 4], f32, isOutput=True)
    bounce_h = nc.dram_tensor("bounce", [n_img * NBLK * NID], f32)

    with tile.TileContext(nc) as tc:
        import contextlib
        ctx = contextlib.ExitStack()
        with ctx:
            cpool = ctx.enter_context(tc.tile_pool(name="consts", bufs=1))
            bandA = ctx.enter_context(tc.tile_pool(name="bandA", bufs=3))
            bandB = ctx.enter_context(tc.tile_pool(name="bandB", bufs=3))
            batchp = ctx.enter_context(tc.tile_pool(name="batchp", bufs=4))
            accp = ctx.enter_context(tc.tile_pool(name="acc", bufs=1))
            psum = ctx.enter_context(tc.tile_pool(name="psum", bufs=4, space="PSUM"))
            fin = ctx.enter_context(tc.tile_pool(name="fin", bufs=1))

            # ---- constants (transposed step-1 bf16 layouts; values bf16-exact)
            iota32_t = cpool.tile([P, NLO * G], bf16)
            nc.gpsimd.iota(iota32_t[:].rearrange("p (i c) -> p i c", i=NLO),
                           pattern=[[1, NLO], [0, G]], base=0, channel_multiplier=0,
                           allow_small_or_imprecise_dtypes=True)
            iota12_t = cpool.tile([P, NHI * G], bf16)
            nc.gpsimd.iota(iota12_t[:].rearrange("p (j c) -> p j c", j=NHI),
                           pattern=[[1, NHI], [0, G]], base=0, channel_multiplier=0,
                           allow_small_or_imprecise_dtypes=True)
            OCTW = min(OCT, W)
            xr_t = cpool.tile([P, NHI * OCTW], bf16)
            nc.gpsimd.iota(xr_t[:].rearrange("p (j c) -> p j c", j=NHI),
                           pattern=[[0, NHI], [1, OCTW]], base=0, channel_multiplier=0,
                           allow_small_or_imprecise_dtypes=True)
            p_col = cpool.tile([P, 1], f32)
            nc.gpsimd.iota(p_col[:], pattern=[[0, 1]], base=0, channel_multiplier=1,
                           allow_small_or_imprecise_dtypes=True)
            ones_col = cpool.tile([P, 1], f32)
            nc.vector.memset(ones_col[:], 1.0)
            hsc_col = cpool.tile([P, 1], f32)
            nc.vector.memset(hsc_col[:], 1.0 / NLO)
            hbi_col = cpool.tile([P, 1], f32)
            nc.vector.memset(hbi_col[:], cast_bias / NLO)
            # id layout after bounce reload: id = 3*p + j at [p, img*3 + j]
            idf = cpool.tile([P, n_img * 3], f32)
            nc.gpsimd.iota(idf[:].rearrange("p (g i) -> p g i", g=n_img),
                           pattern=[[0, n_img], [1, 3]], base=0,
                           channel_multiplier=3,
                           allow_small_or_imprecise_dtypes=True)

            exp_accs = accp.tile([P, n_btiles], f32)
            octw = cpool.tile([P, NOCT], f32)
            nc.gpsimd.iota(octw[:], pattern=[[1, NOCT]], base=0,
                           channel_multiplier=0,
                           allow_small_or_imprecise_dtypes=True)
            nc.vector.tensor_scalar(out=octw[:], in0=octw[:], scalar1=float(OCT),
                                    scalar2=None, op0=Alu.mult)
            xw_scr = accp.tile([NHI, NOCT * NLO], f32)
            xw_dummy = accp.tile([NHI, 1], f32)

            accs = []
            for img in range(n_img):
                a = accp.tile([M, NLO], f32, tag=f"acc{img}")
                cy = accp.tile([NHI, NLO], f32, tag=f"accy{img}")
                cx = accp.tile([NHI, NLO], f32, tag=f"accx{img}")
                nc.vector.memset(a[:], 0.0)
                nc.vector.memset(cy[:], 0.0)
                nc.vector.memset(cx[:], 0.0)
                accs.append((a, cy, cx))

            def store_img(img):
                acc, acc2y, acc2x = accs[img]
                base = img * NBLK * NID
                nc.sync.dma_start(
                    out=bounce_h[base:base + 3 * NID].rearrange("(p c) -> p c", p=M),
                    in_=acc[:])
                nc.sync.dma_start(
                    out=bounce_h[base + 3 * NID:base + 4 * NID]
                    .rearrange("(p c) -> p c", p=NHI), in_=acc2y[:])
                nc.sync.dma_start(
                    out=bounce_h[base + 4 * NID:base + 5 * NID]
                    .rearrange("(p c) -> p c", p=NHI), in_=acc2x[:])

            for img in range(n_img):
                acc, acc2y, acc2x = accs[img]
                for band in range(NB):
                    r0 = band * P
                    label_band = bandA.tile([P, W], i32, tag="label_band")
                    nc.sync.dma_start(out=label_band[:], in_=label_h[img, r0:r0 + P, :])
                    logits_band = bandA.tile([P, W], f32, tag="logits_band")
                    nc.sync.dma_start(out=logits_band[:], in_=logits_h[img, r0:r0 + P, :])

                    # exp + per-partition row-sum fused on ACT
                    exp_scr = bandB.tile([P, W], bf16, tag="exp_scr")
                    nc.scalar.activation(
                        out=exp_scr[:], in_=logits_band[:], func=Act.Exp,
                        accum_out=exp_accs[:, img * NB + band: img * NB + band + 1])

                    # hi = int_cast((label + cast_bias)/32); lo = label - 32*hi
                    hi_i = bandB.tile([P, W], i32, tag="hi_i")
                    nc.scalar.activation(out=hi_i[:], in_=label_band[:],
                                         func=Act.Relu, scale=hsc_col[:, 0:1],
                                         bias=hbi_col[:, 0:1])
                    hi_bf = bandB.tile([P, W], bf16, tag="hi_bf")
                    nc.scalar.activation(out=hi_bf[:], in_=hi_i[:], func=Act.Copy)
                    lo_bf = bandB.tile([P, W], bf16, tag="lo_bf")
                    nc.vector.scalar_tensor_tensor(out=lo_bf[:], in0=hi_bf[:],
                                                   scalar=-float(NLO), in1=label_band[:],
                                                   op0=Alu.mult, op1=Alu.add)

                    ps = psum.tile([M, NOCT * NLO], f32, tag="psband")
                    for oct_i in range(NOCT):
                        for bj in range(BPO):
                            bi = oct_i * BPO + bj
                            c0 = bi * G
                            # transposed layouts: innermost dim = chunk (step 1)
                            alo = batchp.tile([P, NLO * G], bf16, tag="alo")
                            alo_v = alo[:].rearrange("p (i c) -> p i c", i=NLO)
                            lo_b = lo_bf[:, c0:c0 + G].unsqueeze(1).to_broadcast([P, NLO, G])
                            nc.vector.tensor_tensor(
                                out=alo_v, in0=lo_b,
                                in1=iota32_t[:].rearrange("p (i c) -> p i c", i=NLO),
                                op=Alu.is_equal)

                            stat = batchp.tile([P, M * G], bf16, tag="stat")
                            stat_v = stat[:].rearrange("p (s j c) -> p s j c", s=NST, j=NHI)
                            hi_b = hi_bf[:, c0:c0 + G].unsqueeze(1).to_broadcast([P, NHI, G])
                            nc.vector.tensor_tensor(
                                out=stat_v[:, 0, :, :], in0=hi_b,
                                in1=iota12_t[:].rearrange("p (j c) -> p j c", j=NHI),
                                op=Alu.is_equal)
                            # stat * p on ACT (per-partition scale)
                            nc.scalar.activation(out=stat_v[:, 1, :, :],
                                                 in_=stat_v[:, 0, :, :],
                                                 func=Act.Copy, scale=p_col[:, 0:1])
                            # stat * (c mod 256) on GPSIMD
                            xr_sl = xr_t[:].rearrange("p (j c) -> p j c", j=NHI)[
                                :, :, bj * G:(bj + 1) * G]
                            nc.gpsimd.tensor_tensor(out=stat_v[:, 2, :, :],
                                                    in0=stat_v[:, 0, :, :],
                                                    in1=xr_sl, op=Alu.mult)

                            for g in range(G):
                                nc.tensor.matmul(
                                    out=ps[:, oct_i * NLO:(oct_i + 1) * NLO],
                                    lhsT=stat_v[:, :, :, g],
                                    rhs=alo_v[:, :, g],
                                    start=(bj == 0 and g == 0),
                                    stop=(bj == BPO - 1 and g == G - 1),
                                )

                    # evacuate band: band-total = sum over octants (exact f32);
                    # corr_y += 128*band*cnt_band; corr_x += 256*sum_oct oct*cnt_oct
                    ps_v = ps[:].rearrange("m (o i) -> m i o", o=NOCT)
                    bsum = bandB.tile([M, NLO], f32, tag="bsum")
                    nc.vector.tensor_reduce(out=bsum[:], in_=ps_v,
                                            axis=mybir.AxisListType.X, op=Alu.add)
                    nc.vector.tensor_tensor(out=acc[:], in0=acc[:], in1=bsum[:],
                                            op=Alu.add)
                    if band:
                        nc.vector.scalar_tensor_tensor(
                            out=acc2y[:], in0=bsum[0:NHI, :], scalar=float(P * band),
                            in1=acc2y[:], op0=Alu.mult, op1=Alu.add)
                    xw = bandB.tile([NHI, NLO], f32, tag="xw")
                    nc.vector.tensor_tensor(
                        out=xw_scr[:].rearrange("m (o i) -> m i o", o=NOCT),
                        in0=ps_v[0:NHI], in1=octw[0:NHI].unsqueeze(1).to_broadcast(
                            [NHI, NLO, NOCT]),
                        op=Alu.mult)
                    nc.vector.tensor_reduce(
                        out=xw[:], in_=xw_scr[:].rearrange("m (o i) -> m i o", o=NOCT),
                        axis=mybir.AxisListType.X, op=Alu.add)
                    nc.vector.tensor_tensor(out=acc2x[:], in0=acc2x[:], in1=xw[:],
                                            op=Alu.add)
                    if band == NB - 1:
                        store_img(img)

            # ---- finalize ----

            red = fin.tile([P, n_img + 1], f32, tag="red")

            def finalize_img(img):
                base = img * NBLK * NID

                def reload(s):
                    t = fin.tile([P, 3], f32, tag=f"re{s}_{img}")
                    src = bounce_h[base:base + NBLK * NID].rearrange(
                        "(s p j) -> p s j", s=NBLK, p=P)
                    nc.sync.dma_start(out=t[:], in_=src[:, s, :])
                    return t

                cnt = reload(0)
                sy = reload(1)
                sx = reload(2)
                cry = reload(3)
                crx = reload(4)
                nc.vector.tensor_tensor(out=sy[:], in0=sy[:], in1=cry[:], op=Alu.add)
                nc.vector.tensor_tensor(out=sx[:], in0=sx[:], in1=crx[:], op=Alu.add)

                denom = fin.tile([P, 3], f32, tag=f"denom{img}")
                nc.vector.tensor_scalar(out=denom[:], in0=cnt[:], scalar1=1.0,
                                        scalar2=None, op0=Alu.max)
                rcp = fin.tile([P, 3], f32, tag=f"rcp{img}")
                nc.vector.reciprocal(rcp[:], denom[:])

                def floordiv(s_t, nm):
                    # exact floor(s/denom): approximate quotient then +/-1 fix
                    qf = fin.tile([P, 3], f32, tag=f"qf{nm}{img}")
                    nc.vector.tensor_tensor(out=qf[:], in0=s_t[:], in1=rcp[:],
                                            op=Alu.mult)
                    qi = fin.tile([P, 3], i32, tag=f"qi{nm}{img}")
                    nc.vector.tensor_copy(qi[:], qf[:])
                    q = fin.tile([P, 3], f32, tag=f"q{nm}{img}")
                    nc.vector.tensor_copy(q[:], qi[:])
                    r = fin.tile([P, 3], f32, tag=f"r{nm}{img}")
                    nc.vector.tensor_tensor(out=r[:], in0=q[:], in1=denom[:],
                                            op=Alu.mult)
                    nc.vector.tensor_tensor(out=r[:], in0=s_t[:], in1=r[:],
                                            op=Alu.subtract)
                    corr = fin.tile([P, 3], f32, tag=f"corr{nm}{img}")
                    nc.vector.tensor_tensor(out=corr[:], in0=r[:], in1=denom[:],
                                            op=Alu.is_ge)
                    nc.vector.tensor_tensor(out=q[:], in0=q[:], in1=corr[:],
                                            op=Alu.add)
                    nc.vector.tensor_scalar(out=corr[:], in0=r[:], scalar1=0.0,
                                            scalar2=None, op0=Alu.is_lt)
                    nc.vector.tensor_tensor(out=q[:], in0=q[:], in1=corr[:],
                                            op=Alu.subtract)
                    return q

                qy = floordiv(sy, "y")
                qx = floordiv(sx, "x")

                offs_f = fin.tile([P, 3], f32, tag=f"offs_f{img}")
                nc.vector.scalar_tensor_tensor(out=offs_f[:], in0=qy[:],
                                               scalar=float(W), in1=qx[:],
                                               op0=Alu.mult, op1=Alu.add)
                mask = fin.tile([P, 3], f32, tag=f"mask{img}")
                nc.vector.tensor_scalar(out=mask[:], in0=idf[:, img * 3:img * 3 + 3],
                                        scalar1=100.0, scalar2=None, op0=Alu.is_gt)
                m2 = fin.tile([P, 3], f32, tag=f"m2{img}")
                nc.vector.tensor_scalar(out=m2[:], in0=cnt[:], scalar1=0.0,
                                        scalar2=None, op0=Alu.is_gt)
                nc.vector.tensor_tensor(out=mask[:], in0=mask[:], in1=m2[:],
                                        op=Alu.mult)
                nc.vector.tensor_tensor(out=offs_f[:], in0=offs_f[:], in1=mask[:],
                                        op=Alu.mult)
                offs_i = fin.tile([P, 3], i32, tag=f"offs_i{img}")
                nc.vector.tensor_copy(offs_i[:], offs_f[:])

                gath = fin.tile([P, 3], f32, tag=f"gath{img}")
                for j in range(3):
                    nc.gpsimd.indirect_dma_start(
                        out=gath[:, j:j + 1],
                        out_offset=None,
                        in_=logits_h[:].rearrange("i h w -> (i h w)").unsqueeze(1),
                        in_offset=bass.IndirectOffsetOnAxis(
                            ap=offs_i[:, j:j + 1], axis=0),
                        element_offset=img * H * W,
                    )
                nc.vector.tensor_tensor(out=gath[:], in0=gath[:], in1=mask[:],
                                        op=Alu.mult)
                nc.vector.tensor_reduce(out=red[:, img:img + 1], in_=gath[:],
                                        axis=mybir.AxisListType.X, op=Alu.add)

            for img in range(n_img):
                finalize_img(img)
            nc.vector.tensor_reduce(out=red[:, n_img:n_img + 1], in_=exp_accs[:],
                                    axis=mybir.AxisListType.X, op=Alu.add)

            ps_fin = psum.tile([1, n_img + 1], f32, tag="ps_fin")
            nc.tensor.matmul(out=ps_fin[:], lhsT=ones_col[:], rhs=red[:],
                             start=True, stop=True)

            out_sb = fin.tile([1, 4], f32, tag="out_sb")
            nc.vector.memset(out_sb[:], 0.0)
            nc.vector.tensor_copy(out_sb[:, 0:1], ps_fin[:, n_img:n_img + 1])
            for img in range(n_img):
                nc.vector.tensor_copy(out_sb[:, 1 + img:2 + img], ps_fin[:, img:img + 1])
            nc.sync.dma_start(out=out_h[:], in_=out_sb[:])

    nc.compile()
    return nc


_NC_CACHE = {}


def kernel(logits, label):
    logits = np.ascontiguousarray(np.asarray(logits, dtype=np.float32))
    label = np.ascontiguousarray(np.asarray(label, dtype=np.int32))
    assert logits.shape == (B, H, W), logits.shape
    assert label.shape == (B, H, W), label.shape

    from concourse.bass_utils import run_bass_kernel_spmd

    key = (NIMG, H, W)
    if key not in _NC_CACHE:
        _NC_CACHE[key] = _build_nc(NIMG, H, W, G=128)
    nc = _NC_CACHE[key]

    in_maps = [
        {"logits": logits[c * NIMG:(c + 1) * NIMG],
         "label": label[c * NIMG:(c + 1) * NIMG]}
        for c in range(N_CORES)
    ]
    # the axon-proxied device occasionally reports a transient
    # NRT_EXEC_UNIT_UNRECOVERABLE; retry a few times before giving up
    import time as _time
    last_exc = None
    for attempt in range(4):
        try:
            res = run_bass_kernel_spmd(nc, in_maps, list(range(N_CORES)))
            break
        except Exception as e:  # jax.errors.JaxRuntimeError and friends
            last_exc = e
            _time.sleep(2.0 * (attempt + 1))
    else:
        raise last_exc

    # host-side combine of the per-core partial scalars (the two "all-reduces")
    exp_total = 0.0
    inst_total = 0.0
    for c in range(N_CORES):
        o = res.results[c]["out"][0]
        exp_total += float(o[0])
        for i in range(NIMG):
            inst_total += float(o[1 + i])
    int_loss = exp_total / float(B * H * W)
    inst = inst_total / float(B)
    return np.float32(int_loss - inst)

